# revision 39
# baseline (speedup 1.0000x reference)
"""Multi-head causal attention (B=4, T=2048, D=1024, H=16, DH=64) on 8 trn2 cores.

Sharding: core = 2*b + g  (b = batch 0..3, g = head-group 0..1, 8 heads each).
Each core computes q/k/v projections for its 8 heads, causal attention, and the
row-parallel slice of the output projection; the host sums the two partial
outputs per batch and adds the output bias.

v2: single software-pipelined loop — the q/k/v projections for chunk c+1 and
the output projection for chunk c-1 are interleaved (in PE issue order) with
the attention j-loop of chunk c, so ScalarE's exp stream and the DVE
normalization work overlap the projection matmuls instead of running in a
separate phase (429us -> ~285us).  Softmax normalization is per (chunk,
pair): l rows are broadcast first (GPSIMD, base-partition-0 only — writing
at partition offset 64 silently corrupts, hence the quadrant-shift copy),
then inverted with the fast approx reciprocal on all 128 lanes, replacing
the 3.3us single-lane exact reciprocals.  q/k/v PSUM evacuations run on
ScalarE to keep the DVE queue short.  Output is stored bf16 (host sums
partials in fp32).

Per-core dataflow (all matmuls bf16 -> fp32 PSUM):
  xT (D,T) stationary-side input, host pre-transposed, DMA'd in 4 col-chunks
  qT/kT  [2-head pairs, 128 x T]  = Wpair.T @ x.T      (PE, K=128 d-tiles)
  v      [T-tiles 128 x 520]      = x @ Wv (+ ones col per head for row sums)
  ST     [j-tile 128, i-chunk 512] = kT.T @ qT          (K=64, 2 heads packed
                                                         in row groups 0-1/2-3)
  expST  = exp(ST/8)  (ScalarE, scale fused; causal: upper tiles trimmed,
                       diagonal tiles masked with a host 0/1 triangle)
  av     [65, 512] += v_aug.T @ expST  (row 64 = softmax denominator l)
  z      = av[0:64] * (1/l)  (GPSIMD partition_broadcast of l, then 2-ULP
                              reciprocal + multiply on DVE, 128 lanes)
  y      [T x 1024] = concatT.T @ WoT_g slices (K=128 c-tiles, bf16 out)
"""

import numpy as np
import ml_dtypes

import concourse.bass as bass
import concourse.bacc as bacc
import concourse.mybir as mybir
import concourse.tile as tile
from concourse.vector_clock import ScopedClock
from concourse.bass_utils import run_bass_kernel_spmd

BF16 = mybir.dt.bfloat16
F32 = mybir.dt.float32
nbf16 = ml_dtypes.bfloat16

D = 1024
DH = 64
HL = 8          # heads per core
KD = D // 128   # d-tiles


# ---------------------------------------------------------------------------
# Walrus in this build rejects >1 sync-wait on SP TPB_CTRL instructions; split
# the TileContext tail-drain's sem waits into single-wait SP nops.
def _patched_drain_and_barrier(self, tick_clock, wait_clock):
    nc = self.nc
    collector = nc.sync.nop()
    wait_clock.add_sem_waits(
        collector.ins, ScopedClock({None: tick_clock.global_clock})
    )
    si = collector.ins.sync_info
    waits = list(si.on_wait) if si and si.on_wait else []
    if si is not None:
        si.on_wait = waits[:1]
    for w in waits[1:]:
        extra = nc.sync.nop()
        esi = extra.ins.sync_info
        if esi is None:
            extra.ins.sync_info = mybir.SyncInfo(on_wait=[w], on_update=[])
        else:
            esi.on_wait = [w]
    nc.sync.drain()
    nc.all_engine_barrier()
    popped = nc._tile_sem_poison_stack.pop()
    assert popped is self._sem_poison
    nc.clear_and_free_semaphores(list(self.sems.allocated().values()))
    nc.all_engine_barrier()


def _apply_tile_patch():
    tile.TileContext._drain_and_barrier = _patched_drain_and_barrier


# ---------------------------------------------------------------------------
def build_core_program(T=2048, has_bias=False):
    """Causal fast path: one-core program; same NEFF runs SPMD on all 8 cores."""
    import os as _os

    RECIP_MODE = _os.environ.get("K_RECIP", "approx")
    _apply_tile_patch()
    NT = T // 128            # 128-row t-tiles
    CH = min(512, T)         # i-chunk width
    NCH = T // CH            # chunks
    JT_PER_CH = CH // 128    # j-tiles per chunk

    nc = bacc.Bacc("TRN2", target_bir_lowering=False, debug=False)
    xT_d = nc.declare_dram_parameter("xT", [D, T], BF16, isOutput=False)
    wq_d = nc.declare_dram_parameter("wq", [D, 512], BF16, isOutput=False)
    wk_d = nc.declare_dram_parameter("wk", [D, 512], BF16, isOutput=False)
    wv_d = nc.declare_dram_parameter("wv", [D, 512], BF16, isOutput=False)
    wo_d = nc.declare_dram_parameter("wo", [512, D], BF16, isOutput=False)
    tri_d = nc.declare_dram_parameter("tri", [128, 128], BF16, isOutput=False)
    if has_bias:
        wqb_d = nc.declare_dram_parameter("wqb", [1, 512], BF16, isOutput=False)
        wkb_d = nc.declare_dram_parameter("wkb", [1, 512], BF16, isOutput=False)
        wvb_d = nc.declare_dram_parameter("wvb", [1, 512], BF16, isOutput=False)
    y_d = nc.declare_dram_parameter("y", [T, D], BF16, isOutput=True)

    Exp = mybir.ActivationFunctionType.Exp

    with tile.TileContext(nc) as tc:
        with (
            tc.tile_pool(name="singles", bufs=1) as singles,
            tc.tile_pool(name="est", bufs=6) as est_pool,
            tc.tile_pool(name="zp", bufs=8) as zpool,
            tc.tile_pool(name="small", bufs=6) as small,
            tc.tile_pool(name="lb", bufs=3) as lbpool,
            tc.tile_pool(name="yout", bufs=6) as yout,
            tc.tile_pool(name="ps_s", bufs=2, space="PSUM") as ps_s,
            tc.tile_pool(name="ps_av", bufs=2, space="PSUM") as ps_av,
            tc.tile_pool(name="ps_yp", bufs=2, space="PSUM") as ps_yp,
        ):
            # ---- loads (ordered so chunk-0 work can start early) ---------
            xT_sb = singles.tile([128, KD, T], BF16, name="xT_sb")
            wv_sb = singles.tile([128, KD, 512], BF16, name="wv_sb")
            # per-kt first pieces: the first v-proj matmul needs only one
            # kt-slice of wv and 128 columns of one xT slice (~290KB)
            for q in range(KD):
                ds = slice(q * 128, (q + 1) * 128)
                nc.sync.dma_start(out=xT_sb[:, q, 0:128], in_=xT_d[ds, 0:128])
                nc.sync.dma_start(out=wv_sb[:, q, :], in_=wv_d[ds, :])
            tri_sb = singles.tile([128, 128], BF16, name="tri_sb")
            nc.sync.dma_start(out=tri_sb, in_=tri_d[:, :])
            nc.sync.dma_start(
                out=xT_sb[:, :, 128:CH],
                in_=xT_d[:, 128:CH].rearrange("(kt p) t -> p kt t", p=128),
            )
            wq_sb = singles.tile([128, KD, 512], BF16, name="wq_sb")
            nc.sync.dma_start(
                out=wq_sb, in_=wq_d[:, :].rearrange("(kt p) n -> p kt n", p=128)
            )
            wk_sb = singles.tile([128, KD, 512], BF16, name="wk_sb")
            nc.sync.dma_start(
                out=wk_sb, in_=wk_d[:, :].rearrange("(kt p) n -> p kt n", p=128)
            )
            for cc in range(1, NCH):
                nc.sync.dma_start(
                    out=xT_sb[:, :, cc * CH : (cc + 1) * CH],
                    in_=xT_d[:, cc * CH : (cc + 1) * CH].rearrange(
                        "(kt p) t -> p kt t", p=128
                    ),
                )
            wo_sb = singles.tile([128, 4, D], BF16, name="wo_sb")
            nc.sync.dma_start(
                out=wo_sb, in_=wo_d[:, :].rearrange("(ct p) o -> p ct o", p=128)
            )
            if has_bias:
                wqb_sb = singles.tile([1, 512], BF16, name="wqb_sb")
                nc.sync.dma_start(out=wqb_sb, in_=wqb_d[:, :])
                wkb_sb = singles.tile([1, 512], BF16, name="wkb_sb")
                nc.sync.dma_start(out=wkb_sb, in_=wkb_d[:, :])
                wvb_sb = singles.tile([1, 512], BF16, name="wvb_sb")
                nc.sync.dma_start(out=wvb_sb, in_=wvb_d[:, :])
            ones_sb = singles.tile([1, T], BF16, name="ones_sb")
            nc.vector.memset(ones_sb, 1.0)

            # warm the exp activation table while the inputs stream in
            warm_sb = singles.tile([1, 8], F32, name="warm_sb")
            nc.vector.memset(warm_sb, 0.0)
            nc.scalar.activation(warm_sb, warm_sb, Exp, scale=1.0)

            v_sb = singles.tile([128, NT, 8 * 65], BF16, name="v_sb")
            qT_sb = singles.tile([128, 4, T], BF16, name="qT_sb")
            kT_sb = singles.tile([128, 4, T], BF16, name="kT_sb")
            # concat is split by chunk parity so late-chunk normalization
            # writes and earlier-chunk outproj reads live in different tiles
            # (Tile's conservative emission-order dependency tracking would
            # otherwise chain readers to the newest write).
            HT = ((NCH + 1) // 2) * CH
            concat_par = [
                singles.tile([128, 4, HT], BF16, name=f"concat{p}")
                for p in range(2)
            ]

            def concat_of(c):
                return concat_par[c % 2], (c // 2) * CH

            # ---- emission helpers ---------------------------------------
            def emit_v_tile(tt):
                v_ps = ps_yp.tile([128, 512], F32, name="v_ps", tag="yp")
                for kt in range(KD):
                    nc.tensor.matmul(
                        v_ps,
                        xT_sb[:, kt, tt * 128 : (tt + 1) * 128],
                        wv_sb[:, kt, :],
                        start=(kt == 0),
                        stop=(kt == KD - 1 and not has_bias),
                    )
                if has_bias:
                    nc.tensor.matmul(
                        v_ps,
                        ones_sb[0:1, tt * 128 : (tt + 1) * 128],
                        wvb_sb[0:1, :],
                        start=False,
                        stop=True,
                    )
                v_view = v_sb[:, tt, :].rearrange("p (h x) -> p h x", x=65)
                # ScalarE evacuation keeps the DVE queue short so PSUM
                # buffers recycle fast (ACT has slack outside the last chunk)
                nc.scalar.copy(
                    v_view[:, :, 0:64],
                    v_ps.rearrange("p (h x) -> p h x", x=64),
                )
                nc.vector.memset(v_view[:, :, 64:65], 1.0)

            def emit_qk_pair(pr, c):
                cs = slice(c * CH, (c + 1) * CH)
                for which, w_sb, wb_sb, dst in (
                    ("q", wq_sb, wqb_sb if has_bias else None, qT_sb),
                    ("k", wk_sb, wkb_sb if has_bias else None, kT_sb),
                ):
                    qk_ps = ps_yp.tile([128, 512], F32, name="qk_ps", tag="yp")
                    for kt in range(KD):
                        nc.tensor.matmul(
                            qk_ps[:, 0:CH],
                            w_sb[:, kt, pr * 128 : (pr + 1) * 128],
                            xT_sb[:, kt, cs],
                            start=(kt == 0),
                            stop=(kt == KD - 1 and not has_bias),
                        )
                    if has_bias:
                        nc.tensor.matmul(
                            qk_ps[:, 0:CH],
                            wb_sb[0:1, pr * 128 : (pr + 1) * 128],
                            ones_sb[0:1, cs],
                            start=False,
                            stop=True,
                        )
                    nc.scalar.copy(dst[:, pr, cs], qk_ps[:, 0:CH])

            def emit_outproj_group(it, oc, scalar_cast=False, split_dma=False):
                cc_t, cbase = concat_of(it // JT_PER_CH)
                k = it % JT_PER_CH
                y_ps = ps_yp.tile([128, 512], F32, name="y_ps", tag="yp")
                for ct in range(4):
                    nc.tensor.matmul(
                        y_ps,
                        cc_t[:, ct, cbase + k * 128 : cbase + (k + 1) * 128],
                        wo_sb[:, ct, oc * 512 : (oc + 1) * 512],
                        start=(ct == 0),
                        stop=(ct == 3),
                    )
                y_sb = yout.tile([128, 512], BF16, name="y_sb", tag="y")
                if scalar_cast:
                    nc.scalar.copy(y_sb, y_ps)
                else:
                    nc.vector.tensor_copy(y_sb, y_ps)
                ys = y_d[it * 128 : (it + 1) * 128, oc * 512 : (oc + 1) * 512]
                if split_dma:
                    nc.sync.dma_start(out=ys[:, 0:256], in_=y_sb[:, 0:256])
                    nc.sync.dma_start(out=ys[:, 256:512], in_=y_sb[:, 256:512])
                else:
                    nc.sync.dma_start(out=ys, in_=y_sb)

            def emit_outproj_tile(it, scalar_cast=False):
                for oc in range(2):
                    emit_outproj_group(it, oc, scalar_cast=scalar_cast)

            # ---- prologue: projections for chunk 0 ----------------------
            for tt in range(JT_PER_CH):
                emit_v_tile(tt)
            for pr in range(4):
                emit_qk_pair(pr, 0)

            # ---- main pipelined loop ------------------------------------
            for c in range(NCH):
                cs = slice(c * CH, (c + 1) * CH)
                n_j = (c + 1) * JT_PER_CH
                for pr in range(4):
                    av_t = [
                        ps_av.tile([65, 512], F32, name="av", tag="av")
                        for _ in range(2)
                    ]
                    s_tiles = {}

                    def emit_S(J, pr=pr, c=c, s_tiles=s_tiles):
                        r = J - c * JT_PER_CH
                        off = max(0, r) * 128
                        w = CH - off
                        spair = ps_s.tile([128, 1024], F32, name="spair", tag="s")
                        # head A at [off, CH); head B packed at [512, 512+w)
                        # so the exp range [off, 512+w) is gap-free.
                        for hh in range(2):
                            hs = slice(hh * 64, (hh + 1) * 64)
                            dst = (
                                spair[:, off:CH]
                                if hh == 0
                                else spair[:, 512 : 512 + w]
                            )
                            nc.tensor.matmul(
                                dst,
                                kT_sb[hs, pr, J * 128 : (J + 1) * 128],
                                qT_sb[hs, pr, c * CH + off : (c + 1) * CH],
                                start=True,
                                stop=True,
                            )
                        s_tiles[J] = (spair, off)

                    emit_S(0)
                    for J in range(n_j):
                        if J + 1 < n_j:
                            emit_S(J + 1)
                        spair, off = s_tiles.pop(J)
                        w = CH - off
                        r = J - c * JT_PER_CH
                        b_sl = [slice(off, CH), slice(512, 512 + w)]
                        e_pair = est_pool.tile([128, 1024], BF16, name="e_t", tag="e")
                        nc.scalar.activation(
                            e_pair[:, off : 512 + w],
                            spair[:, off : 512 + w],
                            Exp,
                            scale=0.125,
                        )
                        if 0 <= r < JT_PER_CH:
                            for hh in range(2):
                                d0 = b_sl[hh].start
                                nc.vector.tensor_mul(
                                    e_pair[:, d0 : d0 + 128],
                                    e_pair[:, d0 : d0 + 128],
                                    tri_sb,
                                )
                        for hh in range(2):
                            h = 2 * pr + hh
                            nc.tensor.matmul(
                                av_t[hh][:, off:CH],
                                v_sb[:, J, h * 65 : (h + 1) * 65],
                                e_pair[:, b_sl[hh]],
                                start=(J == 0),
                                stop=(J == n_j - 1),
                            )
                    # ---- evacuate + normalize this pair -----------------
                    zpair = zpool.tile([128, 512], BF16, name="zpair", tag="z")
                    lrow = [
                        small.tile([1, 512], F32, name="lrow", tag="l")
                        for _ in range(2)
                    ]
                    # l rows first: the GPSIMD broadcasts (longest part of the
                    # normalization chain) can start while the z casts run
                    for hh in range(2):
                        nc.vector.tensor_copy(lrow[hh], av_t[hh][64:65, :])
                    for hh in range(2):
                        nc.vector.tensor_copy(
                            zpair[hh * 64 : (hh + 1) * 64, :], av_t[hh][0:64, :]
                        )
                    av_t = None
                    # normalization chain part A: broadcasts + reciprocal
                    # (no concat write, so PE work emitted after this picks
                    # up no false dependency on it).  partition_broadcast
                    # writes at base partition 0 only; assemble the
                    # [128,512] tile with a quadrant-shift copy.
                    lbc = lbpool.tile([128, 512], F32, name="lbc", tag="lbc")
                    lbcB = lbpool.tile([64, 512], F32, name="lbcB", tag="lbcB")
                    nc.gpsimd.partition_broadcast(lbc[0:64, :], lrow[0], channels=64)
                    nc.gpsimd.partition_broadcast(lbcB, lrow[1], channels=64)
                    nc.vector.tensor_copy(lbc[64:128, :], lbcB)
                    linv = lbpool.tile([128, 512], F32, name="linv", tag="linv")
                    if RECIP_MODE == "approx":
                        # 51-ULP fast reciprocal is ample for the softmax
                        # denominator (values are O(1..1e3), well-conditioned)
                        nc.vector.reciprocal_approx_fast(linv, lbc)
                    elif RECIP_MODE == "approx2":
                        scratch = lbpool.tile([128, 512], F32, name="lscr", tag="lscr")
                        nc.vector.reciprocal_approx_accurate(linv, lbc, scratch)
                    else:
                        nc.vector.reciprocal(linv, lbc)
                    # PE-side work (projections + outproj) before the concat
                    # mul so it doesn't pick up a conservative dependency on
                    # this pair's concat write.
                    if c + 1 < NCH:
                        emit_qk_pair(pr, c + 1)
                        emit_v_tile((c + 1) * JT_PER_CH + pr)
                        if c > 0:
                            # ScalarE cast: keeps ps_yp recycling off the
                            # DVE norm chain (ACT has slack in chunks 1-2)
                            emit_outproj_tile(
                                (c - 1) * JT_PER_CH + pr, scalar_cast=True
                            )
                    else:
                        # last chunk: hold back two outproj tiles as PE
                        # filler for the final normalization chain (their
                        # reads live in the other parity tile, so no false
                        # dependency on this chunk's concat writes)
                        if pr < 2:
                            emit_outproj_tile((c - 1) * JT_PER_CH + pr)
                        elif pr == 3:
                            emit_outproj_tile((c - 1) * JT_PER_CH + 2)
                            emit_outproj_tile((c - 1) * JT_PER_CH + 3)
                    cc_t, cbase = concat_of(c)
                    nc.vector.tensor_mul(
                        cc_t[:, pr, cbase : cbase + CH], zpair, linv
                    )
            # ---- epilogue: last chunk's output projection ---------------
            for i, it in enumerate(range((NCH - 1) * JT_PER_CH, NCH * JT_PER_CH)):
                last = it == NCH * JT_PER_CH - 1
                for oc in range(2):
                    emit_outproj_group(
                        it,
                        oc,
                        scalar_cast=(oc == 0),
                        split_dma=(last and oc == 1),
                    )
    nc.finalize()
    return nc


# ---------------------------------------------------------------------------
# Fallback for non-causal masks: the original two-phase kernel.
def build_core_program_general(T=2048, mask_mode="causal", has_bias=False):
    """One-core program; same NEFF runs SPMD on all 8 cores."""
    import os as _os

    SKEW = _os.environ.get("K_SKEW", "1") == "1"
    RECIP_MODE = _os.environ.get("K_RECIP", "exact")
    _apply_tile_patch()
    NT = T // 128            # 128-row t-tiles
    CH = min(512, T)         # i-chunk width
    NCH = T // CH            # chunks
    JT_PER_CH = CH // 128    # j-tiles per chunk

    nc = bacc.Bacc("TRN2", target_bir_lowering=False, debug=False)
    xT_d = nc.declare_dram_parameter("xT", [D, T], BF16, isOutput=False)
    wq_d = nc.declare_dram_parameter("wq", [D, 512], BF16, isOutput=False)
    wk_d = nc.declare_dram_parameter("wk", [D, 512], BF16, isOutput=False)
    wv_d = nc.declare_dram_parameter("wv", [D, 512], BF16, isOutput=False)
    wo_d = nc.declare_dram_parameter("wo", [512, D], BF16, isOutput=False)
    tri_d = nc.declare_dram_parameter("tri", [128, 128], BF16, isOutput=False)
    idn_d = nc.declare_dram_parameter("idn", [64, 64], BF16, isOutput=False)
    if mask_mode == "general":
        mt_d = nc.declare_dram_parameter("maskT", [T, T], BF16, isOutput=False)
    if has_bias:
        wqb_d = nc.declare_dram_parameter("wqb", [1, 512], BF16, isOutput=False)
        wkb_d = nc.declare_dram_parameter("wkb", [1, 512], BF16, isOutput=False)
        wvb_d = nc.declare_dram_parameter("wvb", [1, 512], BF16, isOutput=False)
    y_d = nc.declare_dram_parameter("y", [T, D], F32, isOutput=True)

    Exp = mybir.ActivationFunctionType.Exp

    with tile.TileContext(nc) as tc:
        with (
            tc.tile_pool(name="singles", bufs=1) as singles,
            tc.tile_pool(name="est", bufs=4) as est_pool,
            tc.tile_pool(name="small", bufs=6) as small,
            tc.tile_pool(name="yout", bufs=3) as yout,
            tc.tile_pool(name="ps_big", bufs=2, space="PSUM") as ps_big,
            tc.tile_pool(name="ps_av", bufs=2, space="PSUM") as ps_av,
            tc.tile_pool(name="ps_y", bufs=2, space="PSUM") as ps_y,
        ):
            # ---- loads -------------------------------------------------
            xT_sb = singles.tile([128, KD, T], BF16, name="xT_sb")
            nc.sync.dma_start(
                out=xT_sb, in_=xT_d[:, :].rearrange("(kt p) t -> p kt t", p=128)
            )
            wq_sb = singles.tile([128, KD, 512], BF16, name="wq_sb")
            nc.sync.dma_start(
                out=wq_sb, in_=wq_d[:, :].rearrange("(kt p) n -> p kt n", p=128)
            )
            wk_sb = singles.tile([128, KD, 512], BF16, name="wk_sb")
            nc.sync.dma_start(
                out=wk_sb, in_=wk_d[:, :].rearrange("(kt p) n -> p kt n", p=128)
            )
            wv_sb = singles.tile([128, KD, 512], BF16, name="wv_sb")
            nc.sync.dma_start(
                out=wv_sb, in_=wv_d[:, :].rearrange("(kt p) n -> p kt n", p=128)
            )
            wo_sb = singles.tile([128, 4, D], BF16, name="wo_sb")
            nc.sync.dma_start(
                out=wo_sb, in_=wo_d[:, :].rearrange("(ct p) o -> p ct o", p=128)
            )
            tri_sb = singles.tile([128, 128], BF16, name="tri_sb")
            nc.sync.dma_start(out=tri_sb, in_=tri_d[:, :])
            idn_sb = singles.tile([64, 64], BF16, name="idn_sb")
            nc.sync.dma_start(out=idn_sb, in_=idn_d[:, :])
            if has_bias:
                wqb_sb = singles.tile([1, 512], BF16, name="wqb_sb")
                nc.sync.dma_start(out=wqb_sb, in_=wqb_d[:, :])
                wkb_sb = singles.tile([1, 512], BF16, name="wkb_sb")
                nc.sync.dma_start(out=wkb_sb, in_=wkb_d[:, :])
                wvb_sb = singles.tile([1, 512], BF16, name="wvb_sb")
                nc.sync.dma_start(out=wvb_sb, in_=wvb_d[:, :])
                ones_sb = singles.tile([1, T], BF16, name="ones_sb")
                nc.vector.memset(ones_sb, 1.0)

            # ---- v projection: v_sb [t-tile, 8 heads x (64 v + 1 one)] -
            v_sb = singles.tile([128, NT, 8 * 65], BF16, name="v_sb")
            for tt in range(NT):
                v_ps = ps_big.tile([128, 1024], F32, name="v_ps", tag="big")
                for kt in range(KD):
                    nc.tensor.matmul(
                        v_ps[:, 0:512],
                        xT_sb[:, kt, tt * 128 : (tt + 1) * 128],
                        wv_sb[:, kt, :],
                        start=(kt == 0),
                        stop=(kt == KD - 1 and not has_bias),
                    )
                if has_bias:
                    nc.tensor.matmul(
                        v_ps[:, 0:512],
                        ones_sb[0:1, tt * 128 : (tt + 1) * 128],
                        wvb_sb[0:1, :],
                        start=False,
                        stop=True,
                    )
                v_view = v_sb[:, tt, :].rearrange("p (h x) -> p h x", x=65)
                nc.vector.tensor_copy(
                    v_view[:, :, 0:64],
                    v_ps[:, 0:512].rearrange("p (h x) -> p h x", x=64),
                )
                nc.vector.memset(v_view[:, :, 64:65], 1.0)

            # ---- q/k projections: qT/kT [pair, 128(2 heads x 64e), T] --
            qT_sb = singles.tile([128, 4, T], BF16, name="qT_sb")
            kT_sb = singles.tile([128, 4, T], BF16, name="kT_sb")
            for pr in range(4):
                for c in range(NCH):
                    cs = slice(c * CH, (c + 1) * CH)
                    qk_ps = ps_big.tile([128, 1024], F32, name="qk_ps", tag="big")
                    for kt in range(KD):
                        nc.tensor.matmul(
                            qk_ps[:, 0:CH],
                            wq_sb[:, kt, pr * 128 : (pr + 1) * 128],
                            xT_sb[:, kt, cs],
                            start=(kt == 0),
                            stop=(kt == KD - 1 and not has_bias),
                        )
                    if has_bias:
                        nc.tensor.matmul(
                            qk_ps[:, 0:CH],
                            wqb_sb[0:1, pr * 128 : (pr + 1) * 128],
                            ones_sb[0:1, cs],
                            start=False,
                            stop=True,
                        )
                    for kt in range(KD):
                        nc.tensor.matmul(
                            qk_ps[:, 512 : 512 + CH],
                            wk_sb[:, kt, pr * 128 : (pr + 1) * 128],
                            xT_sb[:, kt, cs],
                            start=(kt == 0),
                            stop=(kt == KD - 1 and not has_bias),
                        )
                    if has_bias:
                        nc.tensor.matmul(
                            qk_ps[:, 512 : 512 + CH],
                            wkb_sb[0:1, pr * 128 : (pr + 1) * 128],
                            ones_sb[0:1, cs],
                            start=False,
                            stop=True,
                        )
                    nc.vector.tensor_copy(qT_sb[:, pr, cs], qk_ps[:, 0:CH])
                    nc.vector.tensor_copy(kT_sb[:, pr, cs], qk_ps[:, 512 : 512 + CH])

            # ---- attention + output projection, chunk by chunk ---------
            concat_sb = singles.tile([128, 4, T], BF16, name="concat_sb")
            if mask_mode == "general":
                _mt_cm = tc.tile_pool(name="mtiles", bufs=NT + 2)
                mt_pool = _mt_cm.__enter__()

            def emit_outproj_tile(it):
                y_sb = yout.tile([128, D], F32, name="y_sb", tag="y")
                for oc in range(2):
                    y_ps = ps_y.tile([128, 512], F32, name="y_ps", tag="y")
                    for ct in range(4):
                        nc.tensor.matmul(
                            y_ps,
                            concat_sb[:, ct, it * 128 : (it + 1) * 128],
                            wo_sb[:, ct, oc * 512 : (oc + 1) * 512],
                            start=(ct == 0),
                            stop=(ct == 3),
                        )
                    nc.vector.tensor_copy(y_sb[:, oc * 512 : (oc + 1) * 512], y_ps)
                nc.sync.dma_start(out=y_d[it * 128 : (it + 1) * 128, :], in_=y_sb)

            for c in range(NCH):
                cs = slice(c * CH, (c + 1) * CH)
                n_j = (c + 1) * JT_PER_CH if mask_mode == "causal" else NT
                if mask_mode == "general":
                    m_tiles = []
                    for J in range(n_j):
                        mt = mt_pool.tile([128, 512], BF16, name="mt", tag="mt")
                        nc.sync.dma_start(
                            out=mt[:, :CH],
                            in_=mt_d[J * 128 : (J + 1) * 128, cs],
                        )
                        m_tiles.append(mt)
                for pr in range(4):
                    av_t = [
                        ps_av.tile([65, 512], F32, name="av", tag="av")
                        for _ in range(2)
                    ]
                    s_tiles = {}

                    def emit_S(J, pr=pr, c=c, s_tiles=s_tiles):
                        r = J - c * JT_PER_CH
                        off = max(0, r) * 128 if mask_mode == "causal" else 0
                        w = CH - off
                        spair = ps_big.tile([128, 1024], F32, name="spair", tag="big")
                        for hh in range(2):
                            hs = slice(hh * 64, (hh + 1) * 64)
                            dst = (
                                spair[:, off:CH]
                                if hh == 0
                                else spair[:, 512 : 512 + w]
                            )
                            nc.tensor.matmul(
                                dst,
                                kT_sb[hs, pr, J * 128 : (J + 1) * 128],
                                qT_sb[hs, pr, c * CH + off : (c + 1) * CH],
                                start=True,
                                stop=True,
                            )
                        s_tiles[J] = (spair, off)

                    if SKEW:
                        emit_S(0)
                    for J in range(n_j):
                        if SKEW:
                            if J + 1 < n_j:
                                emit_S(J + 1)
                        else:
                            emit_S(J)
                        spair, off = s_tiles.pop(J)
                        w = CH - off
                        r = J - c * JT_PER_CH
                        b_sl = [slice(off, CH), slice(512, 512 + w)]
                        e_pair = est_pool.tile([128, 1024], BF16, name="e_t", tag="e")
                        nc.scalar.activation(
                            e_pair[:, off : 512 + w],
                            spair[:, off : 512 + w],
                            Exp,
                            scale=0.125,
                        )
                        if mask_mode == "causal" and 0 <= r < JT_PER_CH:
                            for hh in range(2):
                                d0 = b_sl[hh].start
                                nc.vector.tensor_mul(
                                    e_pair[:, d0 : d0 + 128],
                                    e_pair[:, d0 : d0 + 128],
                                    tri_sb,
                                )
                        elif mask_mode == "general":
                            for hh in range(2):
                                nc.vector.tensor_mul(
                                    e_pair[:, b_sl[hh]],
                                    e_pair[:, b_sl[hh]],
                                    m_tiles[J][:, :CH],
                                )
                        for hh in range(2):
                            h = 2 * pr + hh
                            nc.tensor.matmul(
                                av_t[hh][:, off:CH],
                                v_sb[:, J, h * 65 : (h + 1) * 65],
                                e_pair[:, b_sl[hh]],
                                start=(J == 0),
                                stop=(J == n_j - 1),
                            )
                    for hh in range(2):
                        hs = slice(hh * 64, (hh + 1) * 64)
                        av = av_t[hh]
                        l_sb = small.tile([1, 512], F32, name="l_sb", tag="lsb")
                        nc.vector.tensor_copy(l_sb[:, :CH], av[64:65, :CH])
                        zraw = small.tile([64, 512], BF16, name="zraw", tag="zraw")
                        nc.vector.tensor_copy(zraw[:, :CH], av[0:64, :CH])
                        av = None
                        linv = small.tile([1, 512], F32, name="linv", tag="linv")
                        if RECIP_MODE == "approx":
                            nc.vector.reciprocal_approx_fast(
                                linv[:, :CH], l_sb[:, :CH]
                            )
                        elif RECIP_MODE == "lnexp":
                            lt = small.tile([1, 512], F32, name="lt", tag="lt")
                            nc.scalar.activation(
                                lt[:, :CH],
                                l_sb[:, :CH],
                                mybir.ActivationFunctionType.Ln,
                                scale=1.0,
                            )
                            nc.scalar.activation(
                                linv[:, :CH],
                                lt[:, :CH],
                                Exp,
                                scale=-1.0,
                            )
                        else:
                            nc.vector.reciprocal(linv[:, :CH], l_sb[:, :CH])
                        lbc = small.tile([64, 512], F32, name="lbc", tag="lbc")
                        nc.gpsimd.partition_broadcast(
                            lbc[:, :CH], linv[:, :CH], channels=64
                        )
                        nc.vector.tensor_mul(
                            concat_sb[hs, pr, cs], zraw[:, :CH], lbc[:, :CH]
                        )
                    if c > 0:
                        emit_outproj_tile((c - 1) * JT_PER_CH + pr)
                for it in range((NCH - 1) * JT_PER_CH, NCH * JT_PER_CH):
                    if c == NCH - 1:
                        emit_outproj_tile(it)
            if mask_mode == "general":
                _mt_cm.__exit__(None, None, None)
    nc.finalize()
    return nc


# ---------------------------------------------------------------------------
# Optional NTFF profiling (test.py sets TRACE=True). Registers the missing
# antenv.axon_hooks module so run_bass_kernel_spmd's trace path works.
TRACE = False
LAST_EXEC_TIME_NS = None
LAST_RESULTS = None


def _ensure_ntff_hook():
    import sys as _sys
    import types as _types

    if "antenv.axon_hooks" in _sys.modules:
        return
    mod = _types.ModuleType("antenv.axon_hooks")
    state = {"hook": None}
    mod.set_axon_ntff_profile_hook = lambda h: state.__setitem__("hook", h)
    mod.get_axon_ntff_profile_hook = lambda: state["hook"]
    _sys.modules["antenv.axon_hooks"] = mod
    import antenv

    antenv.axon_hooks = mod
    try:
        from trn_agent_boot.trn_boot import _ntff_profile_via_ctypes

        hook = _ntff_profile_via_ctypes("/opt/axon/libaxon_pjrt.so")
        if hook is not None:
            mod.set_axon_ntff_profile_hook(hook)
    except Exception:
        pass


_PROGRAM_CACHE = {}


def _get_program(T, mask_mode, has_bias):
    key = (T, mask_mode, has_bias)
    if key not in _PROGRAM_CACHE:
        if mask_mode == "causal":
            _PROGRAM_CACHE[key] = build_core_program(T, has_bias)
        else:
            _PROGRAM_CACHE[key] = build_core_program_general(T, mask_mode, has_bias)
    return _PROGRAM_CACHE[key]


def _mask_mode_of(mask):
    m = np.asarray(mask)
    if m.all():
        return "full"
    T = m.shape[0]
    tril = np.tril(np.ones((T, T), dtype=bool))
    if np.array_equal(m.astype(bool), tril):
        return "causal"
    return "general"


def kernel(x, mask, Wq, bq, Wk, bk, Wv, bv, Wo, bo):
    x = np.asarray(x)
    B, T, D_ = x.shape
    H = Wq.shape[0]
    assert D_ == D and H == 16
    mask_mode = _mask_mode_of(mask)
    has_bias = bool(
        np.any(np.asarray(bq)) or np.any(np.asarray(bk)) or np.any(np.asarray(bv))
    )
    nc = _get_program(T, mask_mode, has_bias)

    tri = np.triu(np.ones((128, 128), dtype=np.float32)).astype(nbf16)
    idn = np.eye(64, dtype=np.float32).astype(nbf16)
    if mask_mode == "general":
        maskT = np.ascontiguousarray(np.asarray(mask).T.astype(np.float32)).astype(
            nbf16
        )

    in_maps = []
    for core in range(8):
        b, g = core // 2, core % 2
        hsl = slice(g * HL, (g + 1) * HL)
        # (h, d, e) -> (d, h*e)
        wq = np.ascontiguousarray(
            np.transpose(np.asarray(Wq)[hsl], (1, 0, 2)).reshape(D, 512)
        ).astype(nbf16)
        wk = np.ascontiguousarray(
            np.transpose(np.asarray(Wk)[hsl], (1, 0, 2)).reshape(D, 512)
        ).astype(nbf16)
        wv = np.ascontiguousarray(
            np.transpose(np.asarray(Wv)[hsl], (1, 0, 2)).reshape(D, 512)
        ).astype(nbf16)
        wo = np.ascontiguousarray(np.asarray(Wo)[:, g * 512 : (g + 1) * 512].T).astype(
            nbf16
        )
        im = {
            "xT": np.ascontiguousarray(x[b].T).astype(nbf16),
            "wq": wq,
            "wk": wk,
            "wv": wv,
            "wo": wo,
            "tri": tri,
        }
        if mask_mode != "causal":
            im["idn"] = idn
        if mask_mode == "general":
            im["maskT"] = maskT
        if has_bias:
            im["wqb"] = np.asarray(bq)[hsl].reshape(1, 512).astype(nbf16)
            im["wkb"] = np.asarray(bk)[hsl].reshape(1, 512).astype(nbf16)
            im["wvb"] = np.asarray(bv)[hsl].reshape(1, 512).astype(nbf16)
        in_maps.append(im)

    global LAST_EXEC_TIME_NS, LAST_RESULTS
    if TRACE:
        _ensure_ntff_hook()
    res = run_bass_kernel_spmd(nc, in_maps, core_ids=list(range(8)), trace=TRACE)
    LAST_RESULTS = res
    if TRACE:
        LAST_EXEC_TIME_NS = res.exec_time_ns
    out = np.empty((B, T, D), dtype=np.float32)
    bo_f = np.asarray(bo, dtype=np.float32)
    for b in range(B):
        out[b] = (
            np.asarray(res.results[2 * b]["y"], dtype=np.float32)
            + np.asarray(res.results[2 * b + 1]["y"], dtype=np.float32)
            + bo_f
        )
    return out


# revision 40
# speedup vs baseline: 1.0231x; 1.0231x over previous
"""Multi-head causal attention (B=4, T=2048, D=1024, H=16, DH=64) on 8 trn2 cores.

Sharding: core = 2*b + g  (b = batch 0..3, g = head-group 0..1, 8 heads each).
Each core computes q/k/v projections for its 8 heads, causal attention, and the
row-parallel slice of the output projection; the host sums the two partial
outputs per batch and adds the output bias.

v2: single software-pipelined loop — the q/k/v projections for chunk c+1 and
the output projection for chunk c-1 are interleaved (in PE issue order) with
the attention j-loop of chunk c, so ScalarE's exp stream and the DVE
normalization work overlap the projection matmuls instead of running in a
separate phase (429us -> ~285us).  Softmax normalization is per (chunk,
pair): l rows are broadcast first (GPSIMD, base-partition-0 only — writing
at partition offset 64 silently corrupts, hence the quadrant-shift copy),
then inverted with the fast approx reciprocal on all 128 lanes, replacing
the 3.3us single-lane exact reciprocals.  q/k/v PSUM evacuations run on
ScalarE to keep the DVE queue short.  Output is stored bf16 (host sums
partials in fp32).

Per-core dataflow (all matmuls bf16 -> fp32 PSUM):
  xT (D,T) stationary-side input, host pre-transposed, DMA'd in 4 col-chunks
  qT/kT  [2-head pairs, 128 x T]  = Wpair.T @ x.T      (PE, K=128 d-tiles)
  v      [T-tiles 128 x 520]      = x @ Wv (+ ones col per head for row sums)
  ST     [j-tile 128, i-chunk 512] = kT.T @ qT          (K=64, 2 heads packed
                                                         in row groups 0-1/2-3)
  expST  = exp(ST/8)  (ScalarE, scale fused; causal: upper tiles trimmed,
                       diagonal tiles masked with a host 0/1 triangle)
  av     [65, 512] += v_aug.T @ expST  (row 64 = softmax denominator l)
  z      = av[0:64] * (1/l)  (GPSIMD partition_broadcast of l, then 2-ULP
                              reciprocal + multiply on DVE, 128 lanes)
  y      [T x 1024] = concatT.T @ WoT_g slices (K=128 c-tiles, bf16 out)
"""

import numpy as np
import ml_dtypes

import concourse.bass as bass
import concourse.bacc as bacc
import concourse.mybir as mybir
import concourse.tile as tile
from concourse.vector_clock import ScopedClock
from concourse.bass_utils import run_bass_kernel_spmd

BF16 = mybir.dt.bfloat16
F32 = mybir.dt.float32
nbf16 = ml_dtypes.bfloat16

D = 1024
DH = 64
HL = 8          # heads per core
KD = D // 128   # d-tiles


# ---------------------------------------------------------------------------
# Walrus in this build rejects >1 sync-wait on SP TPB_CTRL instructions; split
# the TileContext tail-drain's sem waits into single-wait SP nops.
def _patched_drain_and_barrier(self, tick_clock, wait_clock):
    nc = self.nc
    collector = nc.sync.nop()
    wait_clock.add_sem_waits(
        collector.ins, ScopedClock({None: tick_clock.global_clock})
    )
    si = collector.ins.sync_info
    waits = list(si.on_wait) if si and si.on_wait else []
    if si is not None:
        si.on_wait = waits[:1]
    for w in waits[1:]:
        extra = nc.sync.nop()
        esi = extra.ins.sync_info
        if esi is None:
            extra.ins.sync_info = mybir.SyncInfo(on_wait=[w], on_update=[])
        else:
            esi.on_wait = [w]
    nc.sync.drain()
    nc.all_engine_barrier()
    popped = nc._tile_sem_poison_stack.pop()
    assert popped is self._sem_poison
    nc.clear_and_free_semaphores(list(self.sems.allocated().values()))
    nc.all_engine_barrier()


def _apply_tile_patch():
    tile.TileContext._drain_and_barrier = _patched_drain_and_barrier


# ---------------------------------------------------------------------------
def build_core_program(T=2048, has_bias=False):
    """Causal fast path: one-core program; same NEFF runs SPMD on all 8 cores."""
    import os as _os

    RECIP_MODE = _os.environ.get("K_RECIP", "approx")
    _apply_tile_patch()
    NT = T // 128            # 128-row t-tiles
    CH = min(512, T)         # i-chunk width
    NCH = T // CH            # chunks
    JT_PER_CH = CH // 128    # j-tiles per chunk

    nc = bacc.Bacc("TRN2", target_bir_lowering=False, debug=False)
    xT_d = nc.declare_dram_parameter("xT", [D, T], BF16, isOutput=False)
    wq_d = nc.declare_dram_parameter("wq", [D, 512], BF16, isOutput=False)
    wk_d = nc.declare_dram_parameter("wk", [D, 512], BF16, isOutput=False)
    wv_d = nc.declare_dram_parameter("wv", [D, 512], BF16, isOutput=False)
    wo_d = nc.declare_dram_parameter("wo", [512, D], BF16, isOutput=False)
    tri_d = nc.declare_dram_parameter("tri", [128, 128], BF16, isOutput=False)
    if has_bias:
        wqb_d = nc.declare_dram_parameter("wqb", [1, 512], BF16, isOutput=False)
        wkb_d = nc.declare_dram_parameter("wkb", [1, 512], BF16, isOutput=False)
        wvb_d = nc.declare_dram_parameter("wvb", [1, 512], BF16, isOutput=False)
    y_d = nc.declare_dram_parameter("y", [T, D], BF16, isOutput=True)

    Exp = mybir.ActivationFunctionType.Exp

    with tile.TileContext(nc) as tc:
        with (
            tc.tile_pool(name="singles", bufs=1) as singles,
            tc.tile_pool(name="est", bufs=6) as est_pool,
            tc.tile_pool(name="zp", bufs=8) as zpool,
            tc.tile_pool(name="small", bufs=6) as small,
            tc.tile_pool(name="lb", bufs=3) as lbpool,
            tc.tile_pool(name="yout", bufs=6) as yout,
            tc.tile_pool(name="ps_s", bufs=2, space="PSUM") as ps_s,
            tc.tile_pool(name="ps_av", bufs=2, space="PSUM") as ps_av,
            tc.tile_pool(name="ps_yp", bufs=2, space="PSUM") as ps_yp,
        ):
            # ---- loads (ordered so chunk-0 work can start early) ---------
            tri_sb = singles.tile([128, 128], BF16, name="tri_sb")
            nc.sync.dma_start(out=tri_sb, in_=tri_d[:, :])
            xT_sb = singles.tile([128, KD, T], BF16, name="xT_sb")
            wv_sb = singles.tile([128, KD, 512], BF16, name="wv_sb")
            for q in range(4):
                ks = slice(q * (KD // 4), (q + 1) * (KD // 4))
                ds = slice(q * (D // 4), (q + 1) * (D // 4))
                nc.sync.dma_start(
                    out=xT_sb[:, ks, 0:128],
                    in_=xT_d[ds, 0:128].rearrange("(kt p) t -> p kt t", p=128),
                )
                nc.sync.dma_start(
                    out=wv_sb[:, ks, :],
                    in_=wv_d[ds, :].rearrange("(kt p) n -> p kt n", p=128),
                )
            nc.sync.dma_start(
                out=xT_sb[:, :, 128:CH],
                in_=xT_d[:, 128:CH].rearrange("(kt p) t -> p kt t", p=128),
            )
            wq_sb = singles.tile([128, KD, 512], BF16, name="wq_sb")
            nc.sync.dma_start(
                out=wq_sb, in_=wq_d[:, :].rearrange("(kt p) n -> p kt n", p=128)
            )
            wk_sb = singles.tile([128, KD, 512], BF16, name="wk_sb")
            nc.sync.dma_start(
                out=wk_sb, in_=wk_d[:, :].rearrange("(kt p) n -> p kt n", p=128)
            )
            for cc in range(1, NCH):
                nc.sync.dma_start(
                    out=xT_sb[:, :, cc * CH : (cc + 1) * CH],
                    in_=xT_d[:, cc * CH : (cc + 1) * CH].rearrange(
                        "(kt p) t -> p kt t", p=128
                    ),
                )
            wo_sb = singles.tile([128, 4, D], BF16, name="wo_sb")
            nc.sync.dma_start(
                out=wo_sb, in_=wo_d[:, :].rearrange("(ct p) o -> p ct o", p=128)
            )
            if has_bias:
                wqb_sb = singles.tile([1, 512], BF16, name="wqb_sb")
                nc.sync.dma_start(out=wqb_sb, in_=wqb_d[:, :])
                wkb_sb = singles.tile([1, 512], BF16, name="wkb_sb")
                nc.sync.dma_start(out=wkb_sb, in_=wkb_d[:, :])
                wvb_sb = singles.tile([1, 512], BF16, name="wvb_sb")
                nc.sync.dma_start(out=wvb_sb, in_=wvb_d[:, :])
            ones_sb = singles.tile([1, T], BF16, name="ones_sb")
            nc.vector.memset(ones_sb, 1.0)

            # warm the exp activation table while the inputs stream in
            warm_sb = singles.tile([1, 8], F32, name="warm_sb")
            nc.vector.memset(warm_sb, 0.0)
            nc.scalar.activation(warm_sb, warm_sb, Exp, scale=1.0)

            v_sb = singles.tile([128, NT, 8 * 65], BF16, name="v_sb")
            qT_sb = singles.tile([128, 4, T], BF16, name="qT_sb")
            kT_sb = singles.tile([128, 4, T], BF16, name="kT_sb")
            # concat is split by chunk parity so late-chunk normalization
            # writes and earlier-chunk outproj reads live in different tiles
            # (Tile's conservative emission-order dependency tracking would
            # otherwise chain readers to the newest write).
            HT = ((NCH + 1) // 2) * CH
            concat_par = [
                singles.tile([128, 4, HT], BF16, name=f"concat{p}")
                for p in range(2)
            ]

            def concat_of(c):
                return concat_par[c % 2], (c // 2) * CH

            # ---- emission helpers ---------------------------------------
            def emit_v_tile(tt):
                v_ps = ps_yp.tile([128, 512], F32, name="v_ps", tag="yp")
                for kt in range(KD):
                    nc.tensor.matmul(
                        v_ps,
                        xT_sb[:, kt, tt * 128 : (tt + 1) * 128],
                        wv_sb[:, kt, :],
                        start=(kt == 0),
                        stop=(kt == KD - 1 and not has_bias),
                    )
                if has_bias:
                    nc.tensor.matmul(
                        v_ps,
                        ones_sb[0:1, tt * 128 : (tt + 1) * 128],
                        wvb_sb[0:1, :],
                        start=False,
                        stop=True,
                    )
                v_view = v_sb[:, tt, :].rearrange("p (h x) -> p h x", x=65)
                # ScalarE evacuation keeps the DVE queue short so PSUM
                # buffers recycle fast (ACT has slack outside the last chunk)
                nc.scalar.copy(
                    v_view[:, :, 0:64],
                    v_ps.rearrange("p (h x) -> p h x", x=64),
                )
                nc.vector.memset(v_view[:, :, 64:65], 1.0)

            def emit_qk_pair(pr, c):
                cs = slice(c * CH, (c + 1) * CH)
                for which, w_sb, wb_sb, dst in (
                    ("q", wq_sb, wqb_sb if has_bias else None, qT_sb),
                    ("k", wk_sb, wkb_sb if has_bias else None, kT_sb),
                ):
                    qk_ps = ps_yp.tile([128, 512], F32, name="qk_ps", tag="yp")
                    for kt in range(KD):
                        nc.tensor.matmul(
                            qk_ps[:, 0:CH],
                            w_sb[:, kt, pr * 128 : (pr + 1) * 128],
                            xT_sb[:, kt, cs],
                            start=(kt == 0),
                            stop=(kt == KD - 1 and not has_bias),
                        )
                    if has_bias:
                        nc.tensor.matmul(
                            qk_ps[:, 0:CH],
                            wb_sb[0:1, pr * 128 : (pr + 1) * 128],
                            ones_sb[0:1, cs],
                            start=False,
                            stop=True,
                        )
                    nc.scalar.copy(dst[:, pr, cs], qk_ps[:, 0:CH])

            def emit_outproj_group(it, oc, scalar_cast=False, split_dma=False):
                cc_t, cbase = concat_of(it // JT_PER_CH)
                k = it % JT_PER_CH
                y_ps = ps_yp.tile([128, 512], F32, name="y_ps", tag="yp")
                for ct in range(4):
                    nc.tensor.matmul(
                        y_ps,
                        cc_t[:, ct, cbase + k * 128 : cbase + (k + 1) * 128],
                        wo_sb[:, ct, oc * 512 : (oc + 1) * 512],
                        start=(ct == 0),
                        stop=(ct == 3),
                    )
                y_sb = yout.tile([128, 512], BF16, name="y_sb", tag="y")
                if scalar_cast:
                    nc.scalar.copy(y_sb, y_ps)
                else:
                    nc.vector.tensor_copy(y_sb, y_ps)
                ys = y_d[it * 128 : (it + 1) * 128, oc * 512 : (oc + 1) * 512]
                if split_dma:
                    nc.sync.dma_start(out=ys[:, 0:256], in_=y_sb[:, 0:256])
                    nc.sync.dma_start(out=ys[:, 256:512], in_=y_sb[:, 256:512])
                else:
                    nc.sync.dma_start(out=ys, in_=y_sb)

            def emit_outproj_tile(it, scalar_cast=False):
                for oc in range(2):
                    emit_outproj_group(it, oc, scalar_cast=scalar_cast)

            # ---- prologue: projections for chunk 0 ----------------------
            for tt in range(JT_PER_CH):
                emit_v_tile(tt)
            for pr in range(4):
                emit_qk_pair(pr, 0)

            # ---- main pipelined loop ------------------------------------
            for c in range(NCH):
                cs = slice(c * CH, (c + 1) * CH)
                n_j = (c + 1) * JT_PER_CH
                for pr in range(4):
                    av_t = [
                        ps_av.tile([65, 512], F32, name="av", tag="av")
                        for _ in range(2)
                    ]
                    s_tiles = {}

                    def emit_S(J, pr=pr, c=c, s_tiles=s_tiles):
                        r = J - c * JT_PER_CH
                        off = max(0, r) * 128
                        w = CH - off
                        spair = ps_s.tile([128, 1024], F32, name="spair", tag="s")
                        # head A at [off, CH); head B packed at [512, 512+w)
                        # so the exp range [off, 512+w) is gap-free.
                        for hh in range(2):
                            hs = slice(hh * 64, (hh + 1) * 64)
                            dst = (
                                spair[:, off:CH]
                                if hh == 0
                                else spair[:, 512 : 512 + w]
                            )
                            nc.tensor.matmul(
                                dst,
                                kT_sb[hs, pr, J * 128 : (J + 1) * 128],
                                qT_sb[hs, pr, c * CH + off : (c + 1) * CH],
                                start=True,
                                stop=True,
                            )
                        s_tiles[J] = (spair, off)

                    emit_S(0)
                    for J in range(n_j):
                        if J + 1 < n_j:
                            emit_S(J + 1)
                        spair, off = s_tiles.pop(J)
                        w = CH - off
                        r = J - c * JT_PER_CH
                        b_sl = [slice(off, CH), slice(512, 512 + w)]
                        e_pair = est_pool.tile([128, 1024], BF16, name="e_t", tag="e")
                        nc.scalar.activation(
                            e_pair[:, off : 512 + w],
                            spair[:, off : 512 + w],
                            Exp,
                            scale=0.125,
                        )
                        if 0 <= r < JT_PER_CH:
                            for hh in range(2):
                                d0 = b_sl[hh].start
                                nc.vector.tensor_mul(
                                    e_pair[:, d0 : d0 + 128],
                                    e_pair[:, d0 : d0 + 128],
                                    tri_sb,
                                )
                        for hh in range(2):
                            h = 2 * pr + hh
                            nc.tensor.matmul(
                                av_t[hh][:, off:CH],
                                v_sb[:, J, h * 65 : (h + 1) * 65],
                                e_pair[:, b_sl[hh]],
                                start=(J == 0),
                                stop=(J == n_j - 1),
                            )
                    # ---- evacuate + normalize this pair -----------------
                    zpair = zpool.tile([128, 512], BF16, name="zpair", tag="z")
                    lrow = [
                        small.tile([1, 512], F32, name="lrow", tag="l")
                        for _ in range(2)
                    ]
                    # l rows first: the GPSIMD broadcasts (longest part of the
                    # normalization chain) can start while the z casts run
                    for hh in range(2):
                        nc.vector.tensor_copy(lrow[hh], av_t[hh][64:65, :])
                    for hh in range(2):
                        nc.vector.tensor_copy(
                            zpair[hh * 64 : (hh + 1) * 64, :], av_t[hh][0:64, :]
                        )
                    av_t = None
                    # normalization chain part A: broadcasts + reciprocal
                    # (no concat write, so PE work emitted after this picks
                    # up no false dependency on it).  partition_broadcast
                    # writes at base partition 0 only; assemble the
                    # [128,512] tile with a quadrant-shift copy.
                    lbc = lbpool.tile([128, 512], F32, name="lbc", tag="lbc")
                    lbcB = lbpool.tile([64, 512], F32, name="lbcB", tag="lbcB")
                    nc.gpsimd.partition_broadcast(lbc[0:64, :], lrow[0], channels=64)
                    nc.gpsimd.partition_broadcast(lbcB, lrow[1], channels=64)
                    nc.vector.tensor_copy(lbc[64:128, :], lbcB)
                    linv = lbpool.tile([128, 512], F32, name="linv", tag="linv")
                    if RECIP_MODE == "approx":
                        # 51-ULP fast reciprocal is ample for the softmax
                        # denominator (values are O(1..1e3), well-conditioned)
                        nc.vector.reciprocal_approx_fast(linv, lbc)
                    elif RECIP_MODE == "approx2":
                        scratch = lbpool.tile([128, 512], F32, name="lscr", tag="lscr")
                        nc.vector.reciprocal_approx_accurate(linv, lbc, scratch)
                    else:
                        nc.vector.reciprocal(linv, lbc)
                    # PE-side work (projections + outproj) before the concat
                    # mul so it doesn't pick up a conservative dependency on
                    # this pair's concat write.
                    if c + 1 < NCH:
                        emit_qk_pair(pr, c + 1)
                        emit_v_tile((c + 1) * JT_PER_CH + pr)
                        if c > 0:
                            # ScalarE cast: keeps ps_yp recycling off the
                            # DVE norm chain (ACT has slack in chunks 1-2)
                            emit_outproj_tile(
                                (c - 1) * JT_PER_CH + pr, scalar_cast=True
                            )
                    else:
                        # last chunk: hold back two outproj tiles as PE
                        # filler for the final normalization chain (their
                        # reads live in the other parity tile, so no false
                        # dependency on this chunk's concat writes)
                        if pr < 2:
                            emit_outproj_tile((c - 1) * JT_PER_CH + pr)
                        elif pr == 3:
                            emit_outproj_tile((c - 1) * JT_PER_CH + 2)
                            emit_outproj_tile((c - 1) * JT_PER_CH + 3)
                    cc_t, cbase = concat_of(c)
                    nc.vector.tensor_mul(
                        cc_t[:, pr, cbase : cbase + CH], zpair, linv
                    )
            # ---- epilogue: last chunk's output projection ---------------
            for i, it in enumerate(range((NCH - 1) * JT_PER_CH, NCH * JT_PER_CH)):
                last = it == NCH * JT_PER_CH - 1
                for oc in range(2):
                    emit_outproj_group(
                        it,
                        oc,
                        scalar_cast=(oc == 0),
                        split_dma=(last and oc == 1),
                    )
    nc.finalize()
    return nc


# ---------------------------------------------------------------------------
# Fallback for non-causal masks: the original two-phase kernel.
def build_core_program_general(T=2048, mask_mode="causal", has_bias=False):
    """One-core program; same NEFF runs SPMD on all 8 cores."""
    import os as _os

    SKEW = _os.environ.get("K_SKEW", "1") == "1"
    RECIP_MODE = _os.environ.get("K_RECIP", "exact")
    _apply_tile_patch()
    NT = T // 128            # 128-row t-tiles
    CH = min(512, T)         # i-chunk width
    NCH = T // CH            # chunks
    JT_PER_CH = CH // 128    # j-tiles per chunk

    nc = bacc.Bacc("TRN2", target_bir_lowering=False, debug=False)
    xT_d = nc.declare_dram_parameter("xT", [D, T], BF16, isOutput=False)
    wq_d = nc.declare_dram_parameter("wq", [D, 512], BF16, isOutput=False)
    wk_d = nc.declare_dram_parameter("wk", [D, 512], BF16, isOutput=False)
    wv_d = nc.declare_dram_parameter("wv", [D, 512], BF16, isOutput=False)
    wo_d = nc.declare_dram_parameter("wo", [512, D], BF16, isOutput=False)
    tri_d = nc.declare_dram_parameter("tri", [128, 128], BF16, isOutput=False)
    idn_d = nc.declare_dram_parameter("idn", [64, 64], BF16, isOutput=False)
    if mask_mode == "general":
        mt_d = nc.declare_dram_parameter("maskT", [T, T], BF16, isOutput=False)
    if has_bias:
        wqb_d = nc.declare_dram_parameter("wqb", [1, 512], BF16, isOutput=False)
        wkb_d = nc.declare_dram_parameter("wkb", [1, 512], BF16, isOutput=False)
        wvb_d = nc.declare_dram_parameter("wvb", [1, 512], BF16, isOutput=False)
    y_d = nc.declare_dram_parameter("y", [T, D], F32, isOutput=True)

    Exp = mybir.ActivationFunctionType.Exp

    with tile.TileContext(nc) as tc:
        with (
            tc.tile_pool(name="singles", bufs=1) as singles,
            tc.tile_pool(name="est", bufs=4) as est_pool,
            tc.tile_pool(name="small", bufs=6) as small,
            tc.tile_pool(name="yout", bufs=3) as yout,
            tc.tile_pool(name="ps_big", bufs=2, space="PSUM") as ps_big,
            tc.tile_pool(name="ps_av", bufs=2, space="PSUM") as ps_av,
            tc.tile_pool(name="ps_y", bufs=2, space="PSUM") as ps_y,
        ):
            # ---- loads -------------------------------------------------
            xT_sb = singles.tile([128, KD, T], BF16, name="xT_sb")
            nc.sync.dma_start(
                out=xT_sb, in_=xT_d[:, :].rearrange("(kt p) t -> p kt t", p=128)
            )
            wq_sb = singles.tile([128, KD, 512], BF16, name="wq_sb")
            nc.sync.dma_start(
                out=wq_sb, in_=wq_d[:, :].rearrange("(kt p) n -> p kt n", p=128)
            )
            wk_sb = singles.tile([128, KD, 512], BF16, name="wk_sb")
            nc.sync.dma_start(
                out=wk_sb, in_=wk_d[:, :].rearrange("(kt p) n -> p kt n", p=128)
            )
            wv_sb = singles.tile([128, KD, 512], BF16, name="wv_sb")
            nc.sync.dma_start(
                out=wv_sb, in_=wv_d[:, :].rearrange("(kt p) n -> p kt n", p=128)
            )
            wo_sb = singles.tile([128, 4, D], BF16, name="wo_sb")
            nc.sync.dma_start(
                out=wo_sb, in_=wo_d[:, :].rearrange("(ct p) o -> p ct o", p=128)
            )
            tri_sb = singles.tile([128, 128], BF16, name="tri_sb")
            nc.sync.dma_start(out=tri_sb, in_=tri_d[:, :])
            idn_sb = singles.tile([64, 64], BF16, name="idn_sb")
            nc.sync.dma_start(out=idn_sb, in_=idn_d[:, :])
            if has_bias:
                wqb_sb = singles.tile([1, 512], BF16, name="wqb_sb")
                nc.sync.dma_start(out=wqb_sb, in_=wqb_d[:, :])
                wkb_sb = singles.tile([1, 512], BF16, name="wkb_sb")
                nc.sync.dma_start(out=wkb_sb, in_=wkb_d[:, :])
                wvb_sb = singles.tile([1, 512], BF16, name="wvb_sb")
                nc.sync.dma_start(out=wvb_sb, in_=wvb_d[:, :])
                ones_sb = singles.tile([1, T], BF16, name="ones_sb")
                nc.vector.memset(ones_sb, 1.0)

            # ---- v projection: v_sb [t-tile, 8 heads x (64 v + 1 one)] -
            v_sb = singles.tile([128, NT, 8 * 65], BF16, name="v_sb")
            for tt in range(NT):
                v_ps = ps_big.tile([128, 1024], F32, name="v_ps", tag="big")
                for kt in range(KD):
                    nc.tensor.matmul(
                        v_ps[:, 0:512],
                        xT_sb[:, kt, tt * 128 : (tt + 1) * 128],
                        wv_sb[:, kt, :],
                        start=(kt == 0),
                        stop=(kt == KD - 1 and not has_bias),
                    )
                if has_bias:
                    nc.tensor.matmul(
                        v_ps[:, 0:512],
                        ones_sb[0:1, tt * 128 : (tt + 1) * 128],
                        wvb_sb[0:1, :],
                        start=False,
                        stop=True,
                    )
                v_view = v_sb[:, tt, :].rearrange("p (h x) -> p h x", x=65)
                nc.vector.tensor_copy(
                    v_view[:, :, 0:64],
                    v_ps[:, 0:512].rearrange("p (h x) -> p h x", x=64),
                )
                nc.vector.memset(v_view[:, :, 64:65], 1.0)

            # ---- q/k projections: qT/kT [pair, 128(2 heads x 64e), T] --
            qT_sb = singles.tile([128, 4, T], BF16, name="qT_sb")
            kT_sb = singles.tile([128, 4, T], BF16, name="kT_sb")
            for pr in range(4):
                for c in range(NCH):
                    cs = slice(c * CH, (c + 1) * CH)
                    qk_ps = ps_big.tile([128, 1024], F32, name="qk_ps", tag="big")
                    for kt in range(KD):
                        nc.tensor.matmul(
                            qk_ps[:, 0:CH],
                            wq_sb[:, kt, pr * 128 : (pr + 1) * 128],
                            xT_sb[:, kt, cs],
                            start=(kt == 0),
                            stop=(kt == KD - 1 and not has_bias),
                        )
                    if has_bias:
                        nc.tensor.matmul(
                            qk_ps[:, 0:CH],
                            wqb_sb[0:1, pr * 128 : (pr + 1) * 128],
                            ones_sb[0:1, cs],
                            start=False,
                            stop=True,
                        )
                    for kt in range(KD):
                        nc.tensor.matmul(
                            qk_ps[:, 512 : 512 + CH],
                            wk_sb[:, kt, pr * 128 : (pr + 1) * 128],
                            xT_sb[:, kt, cs],
                            start=(kt == 0),
                            stop=(kt == KD - 1 and not has_bias),
                        )
                    if has_bias:
                        nc.tensor.matmul(
                            qk_ps[:, 512 : 512 + CH],
                            wkb_sb[0:1, pr * 128 : (pr + 1) * 128],
                            ones_sb[0:1, cs],
                            start=False,
                            stop=True,
                        )
                    nc.vector.tensor_copy(qT_sb[:, pr, cs], qk_ps[:, 0:CH])
                    nc.vector.tensor_copy(kT_sb[:, pr, cs], qk_ps[:, 512 : 512 + CH])

            # ---- attention + output projection, chunk by chunk ---------
            concat_sb = singles.tile([128, 4, T], BF16, name="concat_sb")
            if mask_mode == "general":
                _mt_cm = tc.tile_pool(name="mtiles", bufs=NT + 2)
                mt_pool = _mt_cm.__enter__()

            def emit_outproj_tile(it):
                y_sb = yout.tile([128, D], F32, name="y_sb", tag="y")
                for oc in range(2):
                    y_ps = ps_y.tile([128, 512], F32, name="y_ps", tag="y")
                    for ct in range(4):
                        nc.tensor.matmul(
                            y_ps,
                            concat_sb[:, ct, it * 128 : (it + 1) * 128],
                            wo_sb[:, ct, oc * 512 : (oc + 1) * 512],
                            start=(ct == 0),
                            stop=(ct == 3),
                        )
                    nc.vector.tensor_copy(y_sb[:, oc * 512 : (oc + 1) * 512], y_ps)
                nc.sync.dma_start(out=y_d[it * 128 : (it + 1) * 128, :], in_=y_sb)

            for c in range(NCH):
                cs = slice(c * CH, (c + 1) * CH)
                n_j = (c + 1) * JT_PER_CH if mask_mode == "causal" else NT
                if mask_mode == "general":
                    m_tiles = []
                    for J in range(n_j):
                        mt = mt_pool.tile([128, 512], BF16, name="mt", tag="mt")
                        nc.sync.dma_start(
                            out=mt[:, :CH],
                            in_=mt_d[J * 128 : (J + 1) * 128, cs],
                        )
                        m_tiles.append(mt)
                for pr in range(4):
                    av_t = [
                        ps_av.tile([65, 512], F32, name="av", tag="av")
                        for _ in range(2)
                    ]
                    s_tiles = {}

                    def emit_S(J, pr=pr, c=c, s_tiles=s_tiles):
                        r = J - c * JT_PER_CH
                        off = max(0, r) * 128 if mask_mode == "causal" else 0
                        w = CH - off
                        spair = ps_big.tile([128, 1024], F32, name="spair", tag="big")
                        for hh in range(2):
                            hs = slice(hh * 64, (hh + 1) * 64)
                            dst = (
                                spair[:, off:CH]
                                if hh == 0
                                else spair[:, 512 : 512 + w]
                            )
                            nc.tensor.matmul(
                                dst,
                                kT_sb[hs, pr, J * 128 : (J + 1) * 128],
                                qT_sb[hs, pr, c * CH + off : (c + 1) * CH],
                                start=True,
                                stop=True,
                            )
                        s_tiles[J] = (spair, off)

                    if SKEW:
                        emit_S(0)
                    for J in range(n_j):
                        if SKEW:
                            if J + 1 < n_j:
                                emit_S(J + 1)
                        else:
                            emit_S(J)
                        spair, off = s_tiles.pop(J)
                        w = CH - off
                        r = J - c * JT_PER_CH
                        b_sl = [slice(off, CH), slice(512, 512 + w)]
                        e_pair = est_pool.tile([128, 1024], BF16, name="e_t", tag="e")
                        nc.scalar.activation(
                            e_pair[:, off : 512 + w],
                            spair[:, off : 512 + w],
                            Exp,
                            scale=0.125,
                        )
                        if mask_mode == "causal" and 0 <= r < JT_PER_CH:
                            for hh in range(2):
                                d0 = b_sl[hh].start
                                nc.vector.tensor_mul(
                                    e_pair[:, d0 : d0 + 128],
                                    e_pair[:, d0 : d0 + 128],
                                    tri_sb,
                                )
                        elif mask_mode == "general":
                            for hh in range(2):
                                nc.vector.tensor_mul(
                                    e_pair[:, b_sl[hh]],
                                    e_pair[:, b_sl[hh]],
                                    m_tiles[J][:, :CH],
                                )
                        for hh in range(2):
                            h = 2 * pr + hh
                            nc.tensor.matmul(
                                av_t[hh][:, off:CH],
                                v_sb[:, J, h * 65 : (h + 1) * 65],
                                e_pair[:, b_sl[hh]],
                                start=(J == 0),
                                stop=(J == n_j - 1),
                            )
                    for hh in range(2):
                        hs = slice(hh * 64, (hh + 1) * 64)
                        av = av_t[hh]
                        l_sb = small.tile([1, 512], F32, name="l_sb", tag="lsb")
                        nc.vector.tensor_copy(l_sb[:, :CH], av[64:65, :CH])
                        zraw = small.tile([64, 512], BF16, name="zraw", tag="zraw")
                        nc.vector.tensor_copy(zraw[:, :CH], av[0:64, :CH])
                        av = None
                        linv = small.tile([1, 512], F32, name="linv", tag="linv")
                        if RECIP_MODE == "approx":
                            nc.vector.reciprocal_approx_fast(
                                linv[:, :CH], l_sb[:, :CH]
                            )
                        elif RECIP_MODE == "lnexp":
                            lt = small.tile([1, 512], F32, name="lt", tag="lt")
                            nc.scalar.activation(
                                lt[:, :CH],
                                l_sb[:, :CH],
                                mybir.ActivationFunctionType.Ln,
                                scale=1.0,
                            )
                            nc.scalar.activation(
                                linv[:, :CH],
                                lt[:, :CH],
                                Exp,
                                scale=-1.0,
                            )
                        else:
                            nc.vector.reciprocal(linv[:, :CH], l_sb[:, :CH])
                        lbc = small.tile([64, 512], F32, name="lbc", tag="lbc")
                        nc.gpsimd.partition_broadcast(
                            lbc[:, :CH], linv[:, :CH], channels=64
                        )
                        nc.vector.tensor_mul(
                            concat_sb[hs, pr, cs], zraw[:, :CH], lbc[:, :CH]
                        )
                    if c > 0:
                        emit_outproj_tile((c - 1) * JT_PER_CH + pr)
                for it in range((NCH - 1) * JT_PER_CH, NCH * JT_PER_CH):
                    if c == NCH - 1:
                        emit_outproj_tile(it)
            if mask_mode == "general":
                _mt_cm.__exit__(None, None, None)
    nc.finalize()
    return nc


# ---------------------------------------------------------------------------
# Optional NTFF profiling (test.py sets TRACE=True). Registers the missing
# antenv.axon_hooks module so run_bass_kernel_spmd's trace path works.
TRACE = False
LAST_EXEC_TIME_NS = None
LAST_RESULTS = None


def _ensure_ntff_hook():
    import sys as _sys
    import types as _types

    if "antenv.axon_hooks" in _sys.modules:
        return
    mod = _types.ModuleType("antenv.axon_hooks")
    state = {"hook": None}
    mod.set_axon_ntff_profile_hook = lambda h: state.__setitem__("hook", h)
    mod.get_axon_ntff_profile_hook = lambda: state["hook"]
    _sys.modules["antenv.axon_hooks"] = mod
    import antenv

    antenv.axon_hooks = mod
    try:
        from trn_agent_boot.trn_boot import _ntff_profile_via_ctypes

        hook = _ntff_profile_via_ctypes("/opt/axon/libaxon_pjrt.so")
        if hook is not None:
            mod.set_axon_ntff_profile_hook(hook)
    except Exception:
        pass


_PROGRAM_CACHE = {}


def _get_program(T, mask_mode, has_bias):
    key = (T, mask_mode, has_bias)
    if key not in _PROGRAM_CACHE:
        if mask_mode == "causal":
            _PROGRAM_CACHE[key] = build_core_program(T, has_bias)
        else:
            _PROGRAM_CACHE[key] = build_core_program_general(T, mask_mode, has_bias)
    return _PROGRAM_CACHE[key]


def _mask_mode_of(mask):
    m = np.asarray(mask)
    if m.all():
        return "full"
    T = m.shape[0]
    tril = np.tril(np.ones((T, T), dtype=bool))
    if np.array_equal(m.astype(bool), tril):
        return "causal"
    return "general"


def kernel(x, mask, Wq, bq, Wk, bk, Wv, bv, Wo, bo):
    x = np.asarray(x)
    B, T, D_ = x.shape
    H = Wq.shape[0]
    assert D_ == D and H == 16
    mask_mode = _mask_mode_of(mask)
    has_bias = bool(
        np.any(np.asarray(bq)) or np.any(np.asarray(bk)) or np.any(np.asarray(bv))
    )
    nc = _get_program(T, mask_mode, has_bias)

    tri = np.triu(np.ones((128, 128), dtype=np.float32)).astype(nbf16)
    idn = np.eye(64, dtype=np.float32).astype(nbf16)
    if mask_mode == "general":
        maskT = np.ascontiguousarray(np.asarray(mask).T.astype(np.float32)).astype(
            nbf16
        )

    in_maps = []
    for core in range(8):
        b, g = core // 2, core % 2
        hsl = slice(g * HL, (g + 1) * HL)
        # (h, d, e) -> (d, h*e)
        wq = np.ascontiguousarray(
            np.transpose(np.asarray(Wq)[hsl], (1, 0, 2)).reshape(D, 512)
        ).astype(nbf16)
        wk = np.ascontiguousarray(
            np.transpose(np.asarray(Wk)[hsl], (1, 0, 2)).reshape(D, 512)
        ).astype(nbf16)
        wv = np.ascontiguousarray(
            np.transpose(np.asarray(Wv)[hsl], (1, 0, 2)).reshape(D, 512)
        ).astype(nbf16)
        wo = np.ascontiguousarray(np.asarray(Wo)[:, g * 512 : (g + 1) * 512].T).astype(
            nbf16
        )
        im = {
            "xT": np.ascontiguousarray(x[b].T).astype(nbf16),
            "wq": wq,
            "wk": wk,
            "wv": wv,
            "wo": wo,
            "tri": tri,
        }
        if mask_mode != "causal":
            im["idn"] = idn
        if mask_mode == "general":
            im["maskT"] = maskT
        if has_bias:
            im["wqb"] = np.asarray(bq)[hsl].reshape(1, 512).astype(nbf16)
            im["wkb"] = np.asarray(bk)[hsl].reshape(1, 512).astype(nbf16)
            im["wvb"] = np.asarray(bv)[hsl].reshape(1, 512).astype(nbf16)
        in_maps.append(im)

    global LAST_EXEC_TIME_NS, LAST_RESULTS
    if TRACE:
        _ensure_ntff_hook()
    res = run_bass_kernel_spmd(nc, in_maps, core_ids=list(range(8)), trace=TRACE)
    LAST_RESULTS = res
    if TRACE:
        LAST_EXEC_TIME_NS = res.exec_time_ns
    out = np.empty((B, T, D), dtype=np.float32)
    bo_f = np.asarray(bo, dtype=np.float32)
    for b in range(B):
        out[b] = (
            np.asarray(res.results[2 * b]["y"], dtype=np.float32)
            + np.asarray(res.results[2 * b + 1]["y"], dtype=np.float32)
            + bo_f
        )
    return out


# revision 42
# speedup vs baseline: 1.0234x; 1.0003x over previous
"""Multi-head causal attention (B=4, T=2048, D=1024, H=16, DH=64) on 8 trn2 cores.

Sharding: core = 2*b + g  (b = batch 0..3, g = head-group 0..1, 8 heads each).
Each core computes q/k/v projections for its 8 heads, causal attention, and the
row-parallel slice of the output projection; the host sums the two partial
outputs per batch and adds the output bias.

v2: single software-pipelined loop — the q/k/v projections for chunk c+1 and
the output projection for chunk c-1 are interleaved (in PE issue order) with
the attention j-loop of chunk c, so ScalarE's exp stream and the DVE
normalization work overlap the projection matmuls instead of running in a
separate phase (429us -> ~285us).  Softmax normalization is per (chunk,
pair): l rows are broadcast first (GPSIMD, base-partition-0 only — writing
at partition offset 64 silently corrupts, hence the quadrant-shift copy),
then inverted with the fast approx reciprocal on all 128 lanes, replacing
the 3.3us single-lane exact reciprocals.  q/k/v PSUM evacuations run on
ScalarE to keep the DVE queue short.  Output is stored bf16 (host sums
partials in fp32).

Per-core dataflow (all matmuls bf16 -> fp32 PSUM):
  xT (D,T) stationary-side input, host pre-transposed, DMA'd in 4 col-chunks
  qT/kT  [2-head pairs, 128 x T]  = Wpair.T @ x.T      (PE, K=128 d-tiles)
  v      [T-tiles 128 x 520]      = x @ Wv (+ ones col per head for row sums)
  ST     [j-tile 128, i-chunk 512] = kT.T @ qT          (K=64, 2 heads packed
                                                         in row groups 0-1/2-3)
  expST  = exp(ST/8)  (ScalarE, scale fused; causal: upper tiles trimmed,
                       diagonal tiles masked with a host 0/1 triangle)
  av     [65, 512] += v_aug.T @ expST  (row 64 = softmax denominator l)
  z      = av[0:64] * (1/l)  (GPSIMD partition_broadcast of l, then 2-ULP
                              reciprocal + multiply on DVE, 128 lanes)
  y      [T x 1024] = concatT.T @ WoT_g slices (K=128 c-tiles, bf16 out)
"""

import numpy as np
import ml_dtypes

import concourse.bass as bass
import concourse.bacc as bacc
import concourse.mybir as mybir
import concourse.tile as tile
from concourse.vector_clock import ScopedClock
from concourse.bass_utils import run_bass_kernel_spmd

BF16 = mybir.dt.bfloat16
F32 = mybir.dt.float32
nbf16 = ml_dtypes.bfloat16

D = 1024
DH = 64
HL = 8          # heads per core
KD = D // 128   # d-tiles


# ---------------------------------------------------------------------------
# Walrus in this build rejects >1 sync-wait on SP TPB_CTRL instructions; split
# the TileContext tail-drain's sem waits into single-wait SP nops.
def _patched_drain_and_barrier(self, tick_clock, wait_clock):
    nc = self.nc
    collector = nc.sync.nop()
    wait_clock.add_sem_waits(
        collector.ins, ScopedClock({None: tick_clock.global_clock})
    )
    si = collector.ins.sync_info
    waits = list(si.on_wait) if si and si.on_wait else []
    if si is not None:
        si.on_wait = waits[:1]
    for w in waits[1:]:
        extra = nc.sync.nop()
        esi = extra.ins.sync_info
        if esi is None:
            extra.ins.sync_info = mybir.SyncInfo(on_wait=[w], on_update=[])
        else:
            esi.on_wait = [w]
    nc.sync.drain()
    nc.all_engine_barrier()
    popped = nc._tile_sem_poison_stack.pop()
    assert popped is self._sem_poison
    nc.clear_and_free_semaphores(list(self.sems.allocated().values()))
    nc.all_engine_barrier()


def _apply_tile_patch():
    tile.TileContext._drain_and_barrier = _patched_drain_and_barrier


# ---------------------------------------------------------------------------
def build_core_program(T=2048, has_bias=False):
    """Causal fast path: one-core program; same NEFF runs SPMD on all 8 cores."""
    import os as _os

    RECIP_MODE = _os.environ.get("K_RECIP", "approx")
    _apply_tile_patch()
    NT = T // 128            # 128-row t-tiles
    CH = min(512, T)         # i-chunk width
    NCH = T // CH            # chunks
    JT_PER_CH = CH // 128    # j-tiles per chunk

    nc = bacc.Bacc("TRN2", target_bir_lowering=False, debug=False)
    xT_d = nc.declare_dram_parameter("xT", [D, T], BF16, isOutput=False)
    wq_d = nc.declare_dram_parameter("wq", [D, 512], BF16, isOutput=False)
    wk_d = nc.declare_dram_parameter("wk", [D, 512], BF16, isOutput=False)
    wv_d = nc.declare_dram_parameter("wv", [D, 512], BF16, isOutput=False)
    wo_d = nc.declare_dram_parameter("wo", [512, D], BF16, isOutput=False)
    tri_d = nc.declare_dram_parameter("tri", [128, 128], BF16, isOutput=False)
    if has_bias:
        wqb_d = nc.declare_dram_parameter("wqb", [1, 512], BF16, isOutput=False)
        wkb_d = nc.declare_dram_parameter("wkb", [1, 512], BF16, isOutput=False)
        wvb_d = nc.declare_dram_parameter("wvb", [1, 512], BF16, isOutput=False)
    y_d = nc.declare_dram_parameter("y", [T, D], BF16, isOutput=True)

    Exp = mybir.ActivationFunctionType.Exp

    with tile.TileContext(nc) as tc:
        with (
            tc.tile_pool(name="singles", bufs=1) as singles,
            tc.tile_pool(name="est", bufs=6) as est_pool,
            tc.tile_pool(name="zp", bufs=8) as zpool,
            tc.tile_pool(name="small", bufs=6) as small,
            tc.tile_pool(name="lb", bufs=3) as lbpool,
            tc.tile_pool(name="yout", bufs=6) as yout,
            tc.tile_pool(name="ps_s", bufs=2, space="PSUM") as ps_s,
            tc.tile_pool(name="ps_av", bufs=2, space="PSUM") as ps_av,
            tc.tile_pool(name="ps_yp", bufs=2, space="PSUM") as ps_yp,
        ):
            # ---- loads (ordered so chunk-0 work can start early) ---------
            tri_sb = singles.tile([128, 128], BF16, name="tri_sb")
            nc.sync.dma_start(out=tri_sb, in_=tri_d[:, :])
            xT_sb = singles.tile([128, KD, T], BF16, name="xT_sb")
            wv_sb = singles.tile([128, KD, 512], BF16, name="wv_sb")
            for q in range(4):
                ks = slice(q * (KD // 4), (q + 1) * (KD // 4))
                ds = slice(q * (D // 4), (q + 1) * (D // 4))
                nc.sync.dma_start(
                    out=xT_sb[:, ks, 0:128],
                    in_=xT_d[ds, 0:128].rearrange("(kt p) t -> p kt t", p=128),
                )
                nc.sync.dma_start(
                    out=wv_sb[:, ks, :],
                    in_=wv_d[ds, :].rearrange("(kt p) n -> p kt n", p=128),
                )
            nc.sync.dma_start(
                out=xT_sb[:, :, 128:CH],
                in_=xT_d[:, 128:CH].rearrange("(kt p) t -> p kt t", p=128),
            )
            wq_sb = singles.tile([128, KD, 512], BF16, name="wq_sb")
            nc.sync.dma_start(
                out=wq_sb, in_=wq_d[:, :].rearrange("(kt p) n -> p kt n", p=128)
            )
            wk_sb = singles.tile([128, KD, 512], BF16, name="wk_sb")
            nc.sync.dma_start(
                out=wk_sb, in_=wk_d[:, :].rearrange("(kt p) n -> p kt n", p=128)
            )
            for cc in range(1, NCH):
                nc.sync.dma_start(
                    out=xT_sb[:, :, cc * CH : (cc + 1) * CH],
                    in_=xT_d[:, cc * CH : (cc + 1) * CH].rearrange(
                        "(kt p) t -> p kt t", p=128
                    ),
                )
            wo_sb = singles.tile([128, 4, D], BF16, name="wo_sb")
            nc.sync.dma_start(
                out=wo_sb, in_=wo_d[:, :].rearrange("(ct p) o -> p ct o", p=128)
            )
            if has_bias:
                wqb_sb = singles.tile([1, 512], BF16, name="wqb_sb")
                nc.sync.dma_start(out=wqb_sb, in_=wqb_d[:, :])
                wkb_sb = singles.tile([1, 512], BF16, name="wkb_sb")
                nc.sync.dma_start(out=wkb_sb, in_=wkb_d[:, :])
                wvb_sb = singles.tile([1, 512], BF16, name="wvb_sb")
                nc.sync.dma_start(out=wvb_sb, in_=wvb_d[:, :])
            ones_sb = singles.tile([1, T], BF16, name="ones_sb")
            nc.vector.memset(ones_sb, 1.0)

            # warm the exp activation table while the inputs stream in
            warm_sb = singles.tile([1, 8], F32, name="warm_sb")
            nc.vector.memset(warm_sb, 0.0)
            nc.scalar.activation(warm_sb, warm_sb, Exp, scale=1.0)

            # warm the PE HAM clock gate (idle default is 1.2 GHz; ~3.4us of
            # sustained matmul activity unlocks 2.4 GHz) with throwaway
            # matmuls during the otherwise-idle input-DMA window, so the
            # first real matmuls run at full clock
            ham_sb = singles.tile([128, 128], BF16, name="ham_sb")
            nc.vector.memset(ham_sb, 0.0)
            ham_ps = ps_yp.tile([128, 512], F32, name="ham_ps", tag="yp")
            for _ in range(9):
                nc.tensor.matmul(
                    ham_ps[:, 0:128], ham_sb, ham_sb, start=True, stop=True
                )
                nc.tensor.matmul(
                    ham_ps[:, 128:256], ham_sb, ham_sb, start=True, stop=True
                )
                nc.tensor.matmul(
                    ham_ps[:, 256:384], ham_sb, ham_sb, start=True, stop=True
                )
                nc.tensor.matmul(
                    ham_ps[:, 384:512], ham_sb, ham_sb, start=True, stop=True
                )

            v_sb = singles.tile([128, NT, 8 * 65], BF16, name="v_sb")
            qT_sb = singles.tile([128, 4, T], BF16, name="qT_sb")
            kT_sb = singles.tile([128, 4, T], BF16, name="kT_sb")
            # concat is split by chunk parity so late-chunk normalization
            # writes and earlier-chunk outproj reads live in different tiles
            # (Tile's conservative emission-order dependency tracking would
            # otherwise chain readers to the newest write).
            HT = ((NCH + 1) // 2) * CH
            concat_par = [
                singles.tile([128, 4, HT], BF16, name=f"concat{p}")
                for p in range(2)
            ]

            def concat_of(c):
                return concat_par[c % 2], (c // 2) * CH

            # ---- emission helpers ---------------------------------------
            def emit_v_tile(tt):
                v_ps = ps_yp.tile([128, 512], F32, name="v_ps", tag="yp")
                for kt in range(KD):
                    nc.tensor.matmul(
                        v_ps,
                        xT_sb[:, kt, tt * 128 : (tt + 1) * 128],
                        wv_sb[:, kt, :],
                        start=(kt == 0),
                        stop=(kt == KD - 1 and not has_bias),
                    )
                if has_bias:
                    nc.tensor.matmul(
                        v_ps,
                        ones_sb[0:1, tt * 128 : (tt + 1) * 128],
                        wvb_sb[0:1, :],
                        start=False,
                        stop=True,
                    )
                v_view = v_sb[:, tt, :].rearrange("p (h x) -> p h x", x=65)
                # ScalarE evacuation keeps the DVE queue short so PSUM
                # buffers recycle fast (ACT has slack outside the last chunk)
                nc.scalar.copy(
                    v_view[:, :, 0:64],
                    v_ps.rearrange("p (h x) -> p h x", x=64),
                )
                nc.vector.memset(v_view[:, :, 64:65], 1.0)

            def emit_qk_pair(pr, c):
                cs = slice(c * CH, (c + 1) * CH)
                for which, w_sb, wb_sb, dst in (
                    ("q", wq_sb, wqb_sb if has_bias else None, qT_sb),
                    ("k", wk_sb, wkb_sb if has_bias else None, kT_sb),
                ):
                    qk_ps = ps_yp.tile([128, 512], F32, name="qk_ps", tag="yp")
                    for kt in range(KD):
                        nc.tensor.matmul(
                            qk_ps[:, 0:CH],
                            w_sb[:, kt, pr * 128 : (pr + 1) * 128],
                            xT_sb[:, kt, cs],
                            start=(kt == 0),
                            stop=(kt == KD - 1 and not has_bias),
                        )
                    if has_bias:
                        nc.tensor.matmul(
                            qk_ps[:, 0:CH],
                            wb_sb[0:1, pr * 128 : (pr + 1) * 128],
                            ones_sb[0:1, cs],
                            start=False,
                            stop=True,
                        )
                    nc.scalar.copy(dst[:, pr, cs], qk_ps[:, 0:CH])

            def emit_outproj_group(it, oc, scalar_cast=False, split_dma=False):
                cc_t, cbase = concat_of(it // JT_PER_CH)
                k = it % JT_PER_CH
                y_ps = ps_yp.tile([128, 512], F32, name="y_ps", tag="yp")
                for ct in range(4):
                    nc.tensor.matmul(
                        y_ps,
                        cc_t[:, ct, cbase + k * 128 : cbase + (k + 1) * 128],
                        wo_sb[:, ct, oc * 512 : (oc + 1) * 512],
                        start=(ct == 0),
                        stop=(ct == 3),
                    )
                y_sb = yout.tile([128, 512], BF16, name="y_sb", tag="y")
                if scalar_cast:
                    nc.scalar.copy(y_sb, y_ps)
                else:
                    nc.vector.tensor_copy(y_sb, y_ps)
                ys = y_d[it * 128 : (it + 1) * 128, oc * 512 : (oc + 1) * 512]
                if split_dma:
                    nc.sync.dma_start(out=ys[:, 0:256], in_=y_sb[:, 0:256])
                    nc.sync.dma_start(out=ys[:, 256:512], in_=y_sb[:, 256:512])
                else:
                    nc.sync.dma_start(out=ys, in_=y_sb)

            def emit_outproj_tile(it, scalar_cast=False):
                for oc in range(2):
                    emit_outproj_group(it, oc, scalar_cast=scalar_cast)

            # ---- prologue: projections for chunk 0 ----------------------
            for tt in range(JT_PER_CH):
                emit_v_tile(tt)
            for pr in range(4):
                emit_qk_pair(pr, 0)

            # ---- main pipelined loop ------------------------------------
            for c in range(NCH):
                cs = slice(c * CH, (c + 1) * CH)
                n_j = (c + 1) * JT_PER_CH
                for pr in range(4):
                    av_t = [
                        ps_av.tile([65, 512], F32, name="av", tag="av")
                        for _ in range(2)
                    ]
                    s_tiles = {}

                    def emit_S(J, pr=pr, c=c, s_tiles=s_tiles):
                        r = J - c * JT_PER_CH
                        off = max(0, r) * 128
                        w = CH - off
                        spair = ps_s.tile([128, 1024], F32, name="spair", tag="s")
                        # head A at [off, CH); head B packed at [512, 512+w)
                        # so the exp range [off, 512+w) is gap-free.
                        for hh in range(2):
                            hs = slice(hh * 64, (hh + 1) * 64)
                            dst = (
                                spair[:, off:CH]
                                if hh == 0
                                else spair[:, 512 : 512 + w]
                            )
                            nc.tensor.matmul(
                                dst,
                                kT_sb[hs, pr, J * 128 : (J + 1) * 128],
                                qT_sb[hs, pr, c * CH + off : (c + 1) * CH],
                                start=True,
                                stop=True,
                            )
                        s_tiles[J] = (spair, off)

                    emit_S(0)
                    for J in range(n_j):
                        if J + 1 < n_j:
                            emit_S(J + 1)
                        spair, off = s_tiles.pop(J)
                        w = CH - off
                        r = J - c * JT_PER_CH
                        b_sl = [slice(off, CH), slice(512, 512 + w)]
                        e_pair = est_pool.tile([128, 1024], BF16, name="e_t", tag="e")
                        nc.scalar.activation(
                            e_pair[:, off : 512 + w],
                            spair[:, off : 512 + w],
                            Exp,
                            scale=0.125,
                        )
                        if 0 <= r < JT_PER_CH:
                            for hh in range(2):
                                d0 = b_sl[hh].start
                                nc.vector.tensor_mul(
                                    e_pair[:, d0 : d0 + 128],
                                    e_pair[:, d0 : d0 + 128],
                                    tri_sb,
                                )
                        for hh in range(2):
                            h = 2 * pr + hh
                            nc.tensor.matmul(
                                av_t[hh][:, off:CH],
                                v_sb[:, J, h * 65 : (h + 1) * 65],
                                e_pair[:, b_sl[hh]],
                                start=(J == 0),
                                stop=(J == n_j - 1),
                            )
                    # ---- evacuate + normalize this pair -----------------
                    zpair = zpool.tile([128, 512], BF16, name="zpair", tag="z")
                    lrow = [
                        small.tile([1, 512], F32, name="lrow", tag="l")
                        for _ in range(2)
                    ]
                    # l rows first: the GPSIMD broadcasts (longest part of the
                    # normalization chain) can start while the z casts run
                    for hh in range(2):
                        nc.vector.tensor_copy(lrow[hh], av_t[hh][64:65, :])
                    for hh in range(2):
                        nc.vector.tensor_copy(
                            zpair[hh * 64 : (hh + 1) * 64, :], av_t[hh][0:64, :]
                        )
                    av_t = None
                    # normalization chain part A: broadcasts + reciprocal
                    # (no concat write, so PE work emitted after this picks
                    # up no false dependency on it).  partition_broadcast
                    # writes at base partition 0 only; assemble the
                    # [128,512] tile with a quadrant-shift copy.
                    lbc = lbpool.tile([128, 512], F32, name="lbc", tag="lbc")
                    lbcB = lbpool.tile([64, 512], F32, name="lbcB", tag="lbcB")
                    nc.gpsimd.partition_broadcast(lbc[0:64, :], lrow[0], channels=64)
                    nc.gpsimd.partition_broadcast(lbcB, lrow[1], channels=64)
                    nc.vector.tensor_copy(lbc[64:128, :], lbcB)
                    linv = lbpool.tile([128, 512], F32, name="linv", tag="linv")
                    if RECIP_MODE == "approx":
                        # 51-ULP fast reciprocal is ample for the softmax
                        # denominator (values are O(1..1e3), well-conditioned)
                        nc.vector.reciprocal_approx_fast(linv, lbc)
                    elif RECIP_MODE == "approx2":
                        scratch = lbpool.tile([128, 512], F32, name="lscr", tag="lscr")
                        nc.vector.reciprocal_approx_accurate(linv, lbc, scratch)
                    else:
                        nc.vector.reciprocal(linv, lbc)
                    # PE-side work (projections + outproj) before the concat
                    # mul so it doesn't pick up a conservative dependency on
                    # this pair's concat write.
                    if c + 1 < NCH:
                        emit_qk_pair(pr, c + 1)
                        emit_v_tile((c + 1) * JT_PER_CH + pr)
                        if c > 0:
                            # ScalarE cast: keeps ps_yp recycling off the
                            # DVE norm chain (ACT has slack in chunks 1-2)
                            emit_outproj_tile(
                                (c - 1) * JT_PER_CH + pr, scalar_cast=True
                            )
                    else:
                        # last chunk: hold back two outproj tiles as PE
                        # filler for the final normalization chain (their
                        # reads live in the other parity tile, so no false
                        # dependency on this chunk's concat writes)
                        if pr < 2:
                            emit_outproj_tile((c - 1) * JT_PER_CH + pr)
                        elif pr == 3:
                            emit_outproj_tile((c - 1) * JT_PER_CH + 2)
                            emit_outproj_tile((c - 1) * JT_PER_CH + 3)
                    cc_t, cbase = concat_of(c)
                    nc.vector.tensor_mul(
                        cc_t[:, pr, cbase : cbase + CH], zpair, linv
                    )
            # ---- epilogue: last chunk's output projection ---------------
            for i, it in enumerate(range((NCH - 1) * JT_PER_CH, NCH * JT_PER_CH)):
                last = it == NCH * JT_PER_CH - 1
                for oc in range(2):
                    emit_outproj_group(
                        it,
                        oc,
                        scalar_cast=(oc == 0),
                        split_dma=(last and oc == 1),
                    )
    nc.finalize()
    return nc


# ---------------------------------------------------------------------------
# Fallback for non-causal masks: the original two-phase kernel.
def build_core_program_general(T=2048, mask_mode="causal", has_bias=False):
    """One-core program; same NEFF runs SPMD on all 8 cores."""
    import os as _os

    SKEW = _os.environ.get("K_SKEW", "1") == "1"
    RECIP_MODE = _os.environ.get("K_RECIP", "exact")
    _apply_tile_patch()
    NT = T // 128            # 128-row t-tiles
    CH = min(512, T)         # i-chunk width
    NCH = T // CH            # chunks
    JT_PER_CH = CH // 128    # j-tiles per chunk

    nc = bacc.Bacc("TRN2", target_bir_lowering=False, debug=False)
    xT_d = nc.declare_dram_parameter("xT", [D, T], BF16, isOutput=False)
    wq_d = nc.declare_dram_parameter("wq", [D, 512], BF16, isOutput=False)
    wk_d = nc.declare_dram_parameter("wk", [D, 512], BF16, isOutput=False)
    wv_d = nc.declare_dram_parameter("wv", [D, 512], BF16, isOutput=False)
    wo_d = nc.declare_dram_parameter("wo", [512, D], BF16, isOutput=False)
    tri_d = nc.declare_dram_parameter("tri", [128, 128], BF16, isOutput=False)
    idn_d = nc.declare_dram_parameter("idn", [64, 64], BF16, isOutput=False)
    if mask_mode == "general":
        mt_d = nc.declare_dram_parameter("maskT", [T, T], BF16, isOutput=False)
    if has_bias:
        wqb_d = nc.declare_dram_parameter("wqb", [1, 512], BF16, isOutput=False)
        wkb_d = nc.declare_dram_parameter("wkb", [1, 512], BF16, isOutput=False)
        wvb_d = nc.declare_dram_parameter("wvb", [1, 512], BF16, isOutput=False)
    y_d = nc.declare_dram_parameter("y", [T, D], F32, isOutput=True)

    Exp = mybir.ActivationFunctionType.Exp

    with tile.TileContext(nc) as tc:
        with (
            tc.tile_pool(name="singles", bufs=1) as singles,
            tc.tile_pool(name="est", bufs=4) as est_pool,
            tc.tile_pool(name="small", bufs=6) as small,
            tc.tile_pool(name="yout", bufs=3) as yout,
            tc.tile_pool(name="ps_big", bufs=2, space="PSUM") as ps_big,
            tc.tile_pool(name="ps_av", bufs=2, space="PSUM") as ps_av,
            tc.tile_pool(name="ps_y", bufs=2, space="PSUM") as ps_y,
        ):
            # ---- loads -------------------------------------------------
            xT_sb = singles.tile([128, KD, T], BF16, name="xT_sb")
            nc.sync.dma_start(
                out=xT_sb, in_=xT_d[:, :].rearrange("(kt p) t -> p kt t", p=128)
            )
            wq_sb = singles.tile([128, KD, 512], BF16, name="wq_sb")
            nc.sync.dma_start(
                out=wq_sb, in_=wq_d[:, :].rearrange("(kt p) n -> p kt n", p=128)
            )
            wk_sb = singles.tile([128, KD, 512], BF16, name="wk_sb")
            nc.sync.dma_start(
                out=wk_sb, in_=wk_d[:, :].rearrange("(kt p) n -> p kt n", p=128)
            )
            wv_sb = singles.tile([128, KD, 512], BF16, name="wv_sb")
            nc.sync.dma_start(
                out=wv_sb, in_=wv_d[:, :].rearrange("(kt p) n -> p kt n", p=128)
            )
            wo_sb = singles.tile([128, 4, D], BF16, name="wo_sb")
            nc.sync.dma_start(
                out=wo_sb, in_=wo_d[:, :].rearrange("(ct p) o -> p ct o", p=128)
            )
            tri_sb = singles.tile([128, 128], BF16, name="tri_sb")
            nc.sync.dma_start(out=tri_sb, in_=tri_d[:, :])
            idn_sb = singles.tile([64, 64], BF16, name="idn_sb")
            nc.sync.dma_start(out=idn_sb, in_=idn_d[:, :])
            if has_bias:
                wqb_sb = singles.tile([1, 512], BF16, name="wqb_sb")
                nc.sync.dma_start(out=wqb_sb, in_=wqb_d[:, :])
                wkb_sb = singles.tile([1, 512], BF16, name="wkb_sb")
                nc.sync.dma_start(out=wkb_sb, in_=wkb_d[:, :])
                wvb_sb = singles.tile([1, 512], BF16, name="wvb_sb")
                nc.sync.dma_start(out=wvb_sb, in_=wvb_d[:, :])
                ones_sb = singles.tile([1, T], BF16, name="ones_sb")
                nc.vector.memset(ones_sb, 1.0)

            # ---- v projection: v_sb [t-tile, 8 heads x (64 v + 1 one)] -
            v_sb = singles.tile([128, NT, 8 * 65], BF16, name="v_sb")
            for tt in range(NT):
                v_ps = ps_big.tile([128, 1024], F32, name="v_ps", tag="big")
                for kt in range(KD):
                    nc.tensor.matmul(
                        v_ps[:, 0:512],
                        xT_sb[:, kt, tt * 128 : (tt + 1) * 128],
                        wv_sb[:, kt, :],
                        start=(kt == 0),
                        stop=(kt == KD - 1 and not has_bias),
                    )
                if has_bias:
                    nc.tensor.matmul(
                        v_ps[:, 0:512],
                        ones_sb[0:1, tt * 128 : (tt + 1) * 128],
                        wvb_sb[0:1, :],
                        start=False,
                        stop=True,
                    )
                v_view = v_sb[:, tt, :].rearrange("p (h x) -> p h x", x=65)
                nc.vector.tensor_copy(
                    v_view[:, :, 0:64],
                    v_ps[:, 0:512].rearrange("p (h x) -> p h x", x=64),
                )
                nc.vector.memset(v_view[:, :, 64:65], 1.0)

            # ---- q/k projections: qT/kT [pair, 128(2 heads x 64e), T] --
            qT_sb = singles.tile([128, 4, T], BF16, name="qT_sb")
            kT_sb = singles.tile([128, 4, T], BF16, name="kT_sb")
            for pr in range(4):
                for c in range(NCH):
                    cs = slice(c * CH, (c + 1) * CH)
                    qk_ps = ps_big.tile([128, 1024], F32, name="qk_ps", tag="big")
                    for kt in range(KD):
                        nc.tensor.matmul(
                            qk_ps[:, 0:CH],
                            wq_sb[:, kt, pr * 128 : (pr + 1) * 128],
                            xT_sb[:, kt, cs],
                            start=(kt == 0),
                            stop=(kt == KD - 1 and not has_bias),
                        )
                    if has_bias:
                        nc.tensor.matmul(
                            qk_ps[:, 0:CH],
                            wqb_sb[0:1, pr * 128 : (pr + 1) * 128],
                            ones_sb[0:1, cs],
                            start=False,
                            stop=True,
                        )
                    for kt in range(KD):
                        nc.tensor.matmul(
                            qk_ps[:, 512 : 512 + CH],
                            wk_sb[:, kt, pr * 128 : (pr + 1) * 128],
                            xT_sb[:, kt, cs],
                            start=(kt == 0),
                            stop=(kt == KD - 1 and not has_bias),
                        )
                    if has_bias:
                        nc.tensor.matmul(
                            qk_ps[:, 512 : 512 + CH],
                            wkb_sb[0:1, pr * 128 : (pr + 1) * 128],
                            ones_sb[0:1, cs],
                            start=False,
                            stop=True,
                        )
                    nc.vector.tensor_copy(qT_sb[:, pr, cs], qk_ps[:, 0:CH])
                    nc.vector.tensor_copy(kT_sb[:, pr, cs], qk_ps[:, 512 : 512 + CH])

            # ---- attention + output projection, chunk by chunk ---------
            concat_sb = singles.tile([128, 4, T], BF16, name="concat_sb")
            if mask_mode == "general":
                _mt_cm = tc.tile_pool(name="mtiles", bufs=NT + 2)
                mt_pool = _mt_cm.__enter__()

            def emit_outproj_tile(it):
                y_sb = yout.tile([128, D], F32, name="y_sb", tag="y")
                for oc in range(2):
                    y_ps = ps_y.tile([128, 512], F32, name="y_ps", tag="y")
                    for ct in range(4):
                        nc.tensor.matmul(
                            y_ps,
                            concat_sb[:, ct, it * 128 : (it + 1) * 128],
                            wo_sb[:, ct, oc * 512 : (oc + 1) * 512],
                            start=(ct == 0),
                            stop=(ct == 3),
                        )
                    nc.vector.tensor_copy(y_sb[:, oc * 512 : (oc + 1) * 512], y_ps)
                nc.sync.dma_start(out=y_d[it * 128 : (it + 1) * 128, :], in_=y_sb)

            for c in range(NCH):
                cs = slice(c * CH, (c + 1) * CH)
                n_j = (c + 1) * JT_PER_CH if mask_mode == "causal" else NT
                if mask_mode == "general":
                    m_tiles = []
                    for J in range(n_j):
                        mt = mt_pool.tile([128, 512], BF16, name="mt", tag="mt")
                        nc.sync.dma_start(
                            out=mt[:, :CH],
                            in_=mt_d[J * 128 : (J + 1) * 128, cs],
                        )
                        m_tiles.append(mt)
                for pr in range(4):
                    av_t = [
                        ps_av.tile([65, 512], F32, name="av", tag="av")
                        for _ in range(2)
                    ]
                    s_tiles = {}

                    def emit_S(J, pr=pr, c=c, s_tiles=s_tiles):
                        r = J - c * JT_PER_CH
                        off = max(0, r) * 128 if mask_mode == "causal" else 0
                        w = CH - off
                        spair = ps_big.tile([128, 1024], F32, name="spair", tag="big")
                        for hh in range(2):
                            hs = slice(hh * 64, (hh + 1) * 64)
                            dst = (
                                spair[:, off:CH]
                                if hh == 0
                                else spair[:, 512 : 512 + w]
                            )
                            nc.tensor.matmul(
                                dst,
                                kT_sb[hs, pr, J * 128 : (J + 1) * 128],
                                qT_sb[hs, pr, c * CH + off : (c + 1) * CH],
                                start=True,
                                stop=True,
                            )
                        s_tiles[J] = (spair, off)

                    if SKEW:
                        emit_S(0)
                    for J in range(n_j):
                        if SKEW:
                            if J + 1 < n_j:
                                emit_S(J + 1)
                        else:
                            emit_S(J)
                        spair, off = s_tiles.pop(J)
                        w = CH - off
                        r = J - c * JT_PER_CH
                        b_sl = [slice(off, CH), slice(512, 512 + w)]
                        e_pair = est_pool.tile([128, 1024], BF16, name="e_t", tag="e")
                        nc.scalar.activation(
                            e_pair[:, off : 512 + w],
                            spair[:, off : 512 + w],
                            Exp,
                            scale=0.125,
                        )
                        if mask_mode == "causal" and 0 <= r < JT_PER_CH:
                            for hh in range(2):
                                d0 = b_sl[hh].start
                                nc.vector.tensor_mul(
                                    e_pair[:, d0 : d0 + 128],
                                    e_pair[:, d0 : d0 + 128],
                                    tri_sb,
                                )
                        elif mask_mode == "general":
                            for hh in range(2):
                                nc.vector.tensor_mul(
                                    e_pair[:, b_sl[hh]],
                                    e_pair[:, b_sl[hh]],
                                    m_tiles[J][:, :CH],
                                )
                        for hh in range(2):
                            h = 2 * pr + hh
                            nc.tensor.matmul(
                                av_t[hh][:, off:CH],
                                v_sb[:, J, h * 65 : (h + 1) * 65],
                                e_pair[:, b_sl[hh]],
                                start=(J == 0),
                                stop=(J == n_j - 1),
                            )
                    for hh in range(2):
                        hs = slice(hh * 64, (hh + 1) * 64)
                        av = av_t[hh]
                        l_sb = small.tile([1, 512], F32, name="l_sb", tag="lsb")
                        nc.vector.tensor_copy(l_sb[:, :CH], av[64:65, :CH])
                        zraw = small.tile([64, 512], BF16, name="zraw", tag="zraw")
                        nc.vector.tensor_copy(zraw[:, :CH], av[0:64, :CH])
                        av = None
                        linv = small.tile([1, 512], F32, name="linv", tag="linv")
                        if RECIP_MODE == "approx":
                            nc.vector.reciprocal_approx_fast(
                                linv[:, :CH], l_sb[:, :CH]
                            )
                        elif RECIP_MODE == "lnexp":
                            lt = small.tile([1, 512], F32, name="lt", tag="lt")
                            nc.scalar.activation(
                                lt[:, :CH],
                                l_sb[:, :CH],
                                mybir.ActivationFunctionType.Ln,
                                scale=1.0,
                            )
                            nc.scalar.activation(
                                linv[:, :CH],
                                lt[:, :CH],
                                Exp,
                                scale=-1.0,
                            )
                        else:
                            nc.vector.reciprocal(linv[:, :CH], l_sb[:, :CH])
                        lbc = small.tile([64, 512], F32, name="lbc", tag="lbc")
                        nc.gpsimd.partition_broadcast(
                            lbc[:, :CH], linv[:, :CH], channels=64
                        )
                        nc.vector.tensor_mul(
                            concat_sb[hs, pr, cs], zraw[:, :CH], lbc[:, :CH]
                        )
                    if c > 0:
                        emit_outproj_tile((c - 1) * JT_PER_CH + pr)
                for it in range((NCH - 1) * JT_PER_CH, NCH * JT_PER_CH):
                    if c == NCH - 1:
                        emit_outproj_tile(it)
            if mask_mode == "general":
                _mt_cm.__exit__(None, None, None)
    nc.finalize()
    return nc


# ---------------------------------------------------------------------------
# Optional NTFF profiling (test.py sets TRACE=True). Registers the missing
# antenv.axon_hooks module so run_bass_kernel_spmd's trace path works.
TRACE = False
LAST_EXEC_TIME_NS = None
LAST_RESULTS = None


def _ensure_ntff_hook():
    import sys as _sys
    import types as _types

    if "antenv.axon_hooks" in _sys.modules:
        return
    mod = _types.ModuleType("antenv.axon_hooks")
    state = {"hook": None}
    mod.set_axon_ntff_profile_hook = lambda h: state.__setitem__("hook", h)
    mod.get_axon_ntff_profile_hook = lambda: state["hook"]
    _sys.modules["antenv.axon_hooks"] = mod
    import antenv

    antenv.axon_hooks = mod
    try:
        from trn_agent_boot.trn_boot import _ntff_profile_via_ctypes

        hook = _ntff_profile_via_ctypes("/opt/axon/libaxon_pjrt.so")
        if hook is not None:
            mod.set_axon_ntff_profile_hook(hook)
    except Exception:
        pass


_PROGRAM_CACHE = {}


def _get_program(T, mask_mode, has_bias):
    key = (T, mask_mode, has_bias)
    if key not in _PROGRAM_CACHE:
        if mask_mode == "causal":
            _PROGRAM_CACHE[key] = build_core_program(T, has_bias)
        else:
            _PROGRAM_CACHE[key] = build_core_program_general(T, mask_mode, has_bias)
    return _PROGRAM_CACHE[key]


def _mask_mode_of(mask):
    m = np.asarray(mask)
    if m.all():
        return "full"
    T = m.shape[0]
    tril = np.tril(np.ones((T, T), dtype=bool))
    if np.array_equal(m.astype(bool), tril):
        return "causal"
    return "general"


def kernel(x, mask, Wq, bq, Wk, bk, Wv, bv, Wo, bo):
    x = np.asarray(x)
    B, T, D_ = x.shape
    H = Wq.shape[0]
    assert D_ == D and H == 16
    mask_mode = _mask_mode_of(mask)
    has_bias = bool(
        np.any(np.asarray(bq)) or np.any(np.asarray(bk)) or np.any(np.asarray(bv))
    )
    nc = _get_program(T, mask_mode, has_bias)

    tri = np.triu(np.ones((128, 128), dtype=np.float32)).astype(nbf16)
    idn = np.eye(64, dtype=np.float32).astype(nbf16)
    if mask_mode == "general":
        maskT = np.ascontiguousarray(np.asarray(mask).T.astype(np.float32)).astype(
            nbf16
        )

    in_maps = []
    for core in range(8):
        b, g = core // 2, core % 2
        hsl = slice(g * HL, (g + 1) * HL)
        # (h, d, e) -> (d, h*e)
        wq = np.ascontiguousarray(
            np.transpose(np.asarray(Wq)[hsl], (1, 0, 2)).reshape(D, 512)
        ).astype(nbf16)
        wk = np.ascontiguousarray(
            np.transpose(np.asarray(Wk)[hsl], (1, 0, 2)).reshape(D, 512)
        ).astype(nbf16)
        wv = np.ascontiguousarray(
            np.transpose(np.asarray(Wv)[hsl], (1, 0, 2)).reshape(D, 512)
        ).astype(nbf16)
        wo = np.ascontiguousarray(np.asarray(Wo)[:, g * 512 : (g + 1) * 512].T).astype(
            nbf16
        )
        im = {
            "xT": np.ascontiguousarray(x[b].T).astype(nbf16),
            "wq": wq,
            "wk": wk,
            "wv": wv,
            "wo": wo,
            "tri": tri,
        }
        if mask_mode != "causal":
            im["idn"] = idn
        if mask_mode == "general":
            im["maskT"] = maskT
        if has_bias:
            im["wqb"] = np.asarray(bq)[hsl].reshape(1, 512).astype(nbf16)
            im["wkb"] = np.asarray(bk)[hsl].reshape(1, 512).astype(nbf16)
            im["wvb"] = np.asarray(bv)[hsl].reshape(1, 512).astype(nbf16)
        in_maps.append(im)

    global LAST_EXEC_TIME_NS, LAST_RESULTS
    if TRACE:
        _ensure_ntff_hook()
    res = run_bass_kernel_spmd(nc, in_maps, core_ids=list(range(8)), trace=TRACE)
    LAST_RESULTS = res
    if TRACE:
        LAST_EXEC_TIME_NS = res.exec_time_ns
    out = np.empty((B, T, D), dtype=np.float32)
    bo_f = np.asarray(bo, dtype=np.float32)
    for b in range(B):
        out[b] = (
            np.asarray(res.results[2 * b]["y"], dtype=np.float32)
            + np.asarray(res.results[2 * b + 1]["y"], dtype=np.float32)
            + bo_f
        )
    return out


# revision 46
# speedup vs baseline: 1.0295x; 1.0060x over previous
"""Multi-head causal attention (B=4, T=2048, D=1024, H=16, DH=64) on 8 trn2 cores.

Sharding: core = 2*b + g  (b = batch 0..3, g = head-group 0..1, 8 heads each).
Each core computes q/k/v projections for its 8 heads, causal attention, and the
row-parallel slice of the output projection; the host sums the two partial
outputs per batch and adds the output bias.

v2: single software-pipelined loop — the q/k/v projections for chunk c+1 and
the output projection for chunk c-1 are interleaved (in PE issue order) with
the attention j-loop of chunk c, so ScalarE's exp stream and the DVE
normalization work overlap the projection matmuls instead of running in a
separate phase (429us -> ~285us).  Softmax normalization is per (chunk,
pair): l rows are broadcast first (GPSIMD, base-partition-0 only — writing
at partition offset 64 silently corrupts, hence the quadrant-shift copy),
then inverted with the fast approx reciprocal on all 128 lanes, replacing
the 3.3us single-lane exact reciprocals.  q/k/v PSUM evacuations run on
ScalarE to keep the DVE queue short.  Output is stored bf16 (host sums
partials in fp32).

Per-core dataflow (all matmuls bf16 -> fp32 PSUM):
  xT (D,T) stationary-side input, host pre-transposed, DMA'd in 4 col-chunks
  qT/kT  [2-head pairs, 128 x T]  = Wpair.T @ x.T      (PE, K=128 d-tiles)
  v      [T-tiles 128 x 520]      = x @ Wv (+ ones col per head for row sums)
  ST     [j-tile 128, i-chunk 512] = kT.T @ qT          (K=64, 2 heads packed
                                                         in row groups 0-1/2-3)
  expST  = exp(ST/8)  (ScalarE, scale fused; causal: upper tiles trimmed,
                       diagonal tiles masked with a host 0/1 triangle)
  av     [65, 512] += v_aug.T @ expST  (row 64 = softmax denominator l)
  z      = av[0:64] * (1/l)  (GPSIMD partition_broadcast of l, then 2-ULP
                              reciprocal + multiply on DVE, 128 lanes)
  y      [T x 1024] = concatT.T @ WoT_g slices (K=128 c-tiles, bf16 out)
"""

import numpy as np
import ml_dtypes

import concourse.bass as bass
import concourse.bacc as bacc
import concourse.mybir as mybir
import concourse.tile as tile
from concourse.vector_clock import ScopedClock
from concourse.bass_utils import run_bass_kernel_spmd

BF16 = mybir.dt.bfloat16
F32 = mybir.dt.float32
nbf16 = ml_dtypes.bfloat16

D = 1024
DH = 64
HL = 8          # heads per core
KD = D // 128   # d-tiles


# ---------------------------------------------------------------------------
# Walrus in this build rejects >1 sync-wait on SP TPB_CTRL instructions; split
# the TileContext tail-drain's sem waits into single-wait SP nops.
def _patched_drain_and_barrier(self, tick_clock, wait_clock):
    nc = self.nc
    collector = nc.sync.nop()
    wait_clock.add_sem_waits(
        collector.ins, ScopedClock({None: tick_clock.global_clock})
    )
    si = collector.ins.sync_info
    waits = list(si.on_wait) if si and si.on_wait else []
    if si is not None:
        si.on_wait = waits[:1]
    for w in waits[1:]:
        extra = nc.sync.nop()
        esi = extra.ins.sync_info
        if esi is None:
            extra.ins.sync_info = mybir.SyncInfo(on_wait=[w], on_update=[])
        else:
            esi.on_wait = [w]
    nc.sync.drain()
    nc.all_engine_barrier()
    popped = nc._tile_sem_poison_stack.pop()
    assert popped is self._sem_poison
    nc.clear_and_free_semaphores(list(self.sems.allocated().values()))
    nc.all_engine_barrier()


def _apply_tile_patch():
    tile.TileContext._drain_and_barrier = _patched_drain_and_barrier


# ---------------------------------------------------------------------------
def build_core_program(T=2048, has_bias=False):
    """Causal fast path: one-core program; same NEFF runs SPMD on all 8 cores."""
    import os as _os

    RECIP_MODE = _os.environ.get("K_RECIP", "approx")
    _apply_tile_patch()
    NT = T // 128            # 128-row t-tiles
    CH = min(512, T)         # i-chunk width
    NCH = T // CH            # chunks
    JT_PER_CH = CH // 128    # j-tiles per chunk

    nc = bacc.Bacc("TRN2", target_bir_lowering=False, debug=False)
    xT_d = nc.declare_dram_parameter("xT", [D, T], BF16, isOutput=False)
    wq_d = nc.declare_dram_parameter("wq", [D, 512], BF16, isOutput=False)
    wk_d = nc.declare_dram_parameter("wk", [D, 512], BF16, isOutput=False)
    wv_d = nc.declare_dram_parameter("wv", [D, 512], BF16, isOutput=False)
    wo_d = nc.declare_dram_parameter("wo", [512, D], BF16, isOutput=False)
    tri_d = nc.declare_dram_parameter("tri", [128, 128], BF16, isOutput=False)
    if has_bias:
        wqb_d = nc.declare_dram_parameter("wqb", [1, 512], BF16, isOutput=False)
        wkb_d = nc.declare_dram_parameter("wkb", [1, 512], BF16, isOutput=False)
        wvb_d = nc.declare_dram_parameter("wvb", [1, 512], BF16, isOutput=False)
    y_d = nc.declare_dram_parameter("y", [T, D], BF16, isOutput=True)

    Exp = mybir.ActivationFunctionType.Exp

    with tile.TileContext(nc) as tc:
        with (
            tc.tile_pool(name="singles", bufs=1) as singles,
            tc.tile_pool(name="est", bufs=6) as est_pool,
            tc.tile_pool(name="zp", bufs=8) as zpool,
            tc.tile_pool(name="small", bufs=6) as small,
            tc.tile_pool(name="lb", bufs=3) as lbpool,
            tc.tile_pool(name="yout", bufs=6) as yout,
            tc.tile_pool(name="ps_s", bufs=2, space="PSUM") as ps_s,
            tc.tile_pool(name="ps_av", bufs=2, space="PSUM") as ps_av,
            tc.tile_pool(name="ps_yp", bufs=2, space="PSUM") as ps_yp,
        ):
            # ---- loads (ordered so chunk-0 work can start early) ---------
            tri_sb = singles.tile([128, 128], BF16, name="tri_sb")
            nc.sync.dma_start(out=tri_sb, in_=tri_d[:, :])
            xT_sb = singles.tile([128, KD, T], BF16, name="xT_sb")
            wv_sb = singles.tile([128, KD, 512], BF16, name="wv_sb")
            for q in range(4):
                ks = slice(q * (KD // 4), (q + 1) * (KD // 4))
                ds = slice(q * (D // 4), (q + 1) * (D // 4))
                nc.sync.dma_start(
                    out=xT_sb[:, ks, 0:128],
                    in_=xT_d[ds, 0:128].rearrange("(kt p) t -> p kt t", p=128),
                )
                nc.sync.dma_start(
                    out=wv_sb[:, ks, :],
                    in_=wv_d[ds, :].rearrange("(kt p) n -> p kt n", p=128),
                )
            nc.sync.dma_start(
                out=xT_sb[:, :, 128:CH],
                in_=xT_d[:, 128:CH].rearrange("(kt p) t -> p kt t", p=128),
            )
            wq_sb = singles.tile([128, KD, 512], BF16, name="wq_sb")
            nc.sync.dma_start(
                out=wq_sb, in_=wq_d[:, :].rearrange("(kt p) n -> p kt n", p=128)
            )
            wk_sb = singles.tile([128, KD, 512], BF16, name="wk_sb")
            nc.sync.dma_start(
                out=wk_sb, in_=wk_d[:, :].rearrange("(kt p) n -> p kt n", p=128)
            )
            for cc in range(1, NCH):
                nc.sync.dma_start(
                    out=xT_sb[:, :, cc * CH : (cc + 1) * CH],
                    in_=xT_d[:, cc * CH : (cc + 1) * CH].rearrange(
                        "(kt p) t -> p kt t", p=128
                    ),
                )
            wo_sb = singles.tile([128, 4, D], BF16, name="wo_sb")
            nc.sync.dma_start(
                out=wo_sb, in_=wo_d[:, :].rearrange("(ct p) o -> p ct o", p=128)
            )
            if has_bias:
                wqb_sb = singles.tile([1, 512], BF16, name="wqb_sb")
                nc.sync.dma_start(out=wqb_sb, in_=wqb_d[:, :])
                wkb_sb = singles.tile([1, 512], BF16, name="wkb_sb")
                nc.sync.dma_start(out=wkb_sb, in_=wkb_d[:, :])
                wvb_sb = singles.tile([1, 512], BF16, name="wvb_sb")
                nc.sync.dma_start(out=wvb_sb, in_=wvb_d[:, :])
            ones_sb = singles.tile([1, T], BF16, name="ones_sb")
            nc.vector.memset(ones_sb, 1.0)

            # warm the exp activation table while the inputs stream in
            warm_sb = singles.tile([1, 8], F32, name="warm_sb")
            nc.vector.memset(warm_sb, 0.0)
            nc.scalar.activation(warm_sb, warm_sb, Exp, scale=1.0)

            # warm the PE HAM clock gate (idle default is 1.2 GHz; ~3.4us of
            # sustained matmul activity unlocks 2.4 GHz) with throwaway
            # matmuls during the otherwise-idle input-DMA window, so the
            # first real matmuls run at full clock
            ham_sb = singles.tile([128, 128], BF16, name="ham_sb")
            nc.vector.memset(ham_sb, 0.0)
            ham_ps = ps_yp.tile([128, 512], F32, name="ham_ps", tag="yp")
            for _ in range(9):
                nc.tensor.matmul(
                    ham_ps[:, 0:128], ham_sb, ham_sb, start=True, stop=True
                )
                nc.tensor.matmul(
                    ham_ps[:, 128:256], ham_sb, ham_sb, start=True, stop=True
                )
                nc.tensor.matmul(
                    ham_ps[:, 256:384], ham_sb, ham_sb, start=True, stop=True
                )
                nc.tensor.matmul(
                    ham_ps[:, 384:512], ham_sb, ham_sb, start=True, stop=True
                )

            v_sb = singles.tile([128, NT, 8 * 65], BF16, name="v_sb")
            qT_sb = singles.tile([128, 4, T], BF16, name="qT_sb")
            kT_sb = singles.tile([128, 4, T], BF16, name="kT_sb")
            # concat is split by chunk parity so late-chunk normalization
            # writes and earlier-chunk outproj reads live in different tiles
            # (Tile's conservative emission-order dependency tracking would
            # otherwise chain readers to the newest write).
            HT = ((NCH + 1) // 2) * CH
            concat_par = [
                singles.tile([128, 4, HT], BF16, name=f"concat{p}")
                for p in range(2)
            ]

            def concat_of(c):
                return concat_par[c % 2], (c // 2) * CH

            # ---- emission helpers ---------------------------------------
            def emit_v_tile(tt):
                v_ps = ps_yp.tile([128, 512], F32, name="v_ps", tag="yp")
                for kt in range(KD):
                    nc.tensor.matmul(
                        v_ps,
                        xT_sb[:, kt, tt * 128 : (tt + 1) * 128],
                        wv_sb[:, kt, :],
                        start=(kt == 0),
                        stop=(kt == KD - 1 and not has_bias),
                    )
                if has_bias:
                    nc.tensor.matmul(
                        v_ps,
                        ones_sb[0:1, tt * 128 : (tt + 1) * 128],
                        wvb_sb[0:1, :],
                        start=False,
                        stop=True,
                    )
                v_view = v_sb[:, tt, :].rearrange("p (h x) -> p h x", x=65)
                # ScalarE evacuation keeps the DVE queue short so PSUM
                # buffers recycle fast (ACT has slack outside the last chunk)
                nc.scalar.copy(
                    v_view[:, :, 0:64],
                    v_ps.rearrange("p (h x) -> p h x", x=64),
                )
                nc.vector.memset(v_view[:, :, 64:65], 1.0)

            def emit_qk_pair(pr, c):
                cs = slice(c * CH, (c + 1) * CH)
                for which, w_sb, wb_sb, dst in (
                    ("q", wq_sb, wqb_sb if has_bias else None, qT_sb),
                    ("k", wk_sb, wkb_sb if has_bias else None, kT_sb),
                ):
                    qk_ps = ps_yp.tile([128, 512], F32, name="qk_ps", tag="yp")
                    for kt in range(KD):
                        nc.tensor.matmul(
                            qk_ps[:, 0:CH],
                            w_sb[:, kt, pr * 128 : (pr + 1) * 128],
                            xT_sb[:, kt, cs],
                            start=(kt == 0),
                            stop=(kt == KD - 1 and not has_bias),
                        )
                    if has_bias:
                        nc.tensor.matmul(
                            qk_ps[:, 0:CH],
                            wb_sb[0:1, pr * 128 : (pr + 1) * 128],
                            ones_sb[0:1, cs],
                            start=False,
                            stop=True,
                        )
                    nc.scalar.copy(dst[:, pr, cs], qk_ps[:, 0:CH])

            def emit_outproj_group(it, oc, scalar_cast=False, split_dma=False):
                cc_t, cbase = concat_of(it // JT_PER_CH)
                k = it % JT_PER_CH
                y_ps = ps_yp.tile([128, 512], F32, name="y_ps", tag="yp")
                for ct in range(4):
                    nc.tensor.matmul(
                        y_ps,
                        cc_t[:, ct, cbase + k * 128 : cbase + (k + 1) * 128],
                        wo_sb[:, ct, oc * 512 : (oc + 1) * 512],
                        start=(ct == 0),
                        stop=(ct == 3),
                    )
                y_sb = yout.tile([128, 512], BF16, name="y_sb", tag="y")
                if scalar_cast:
                    nc.scalar.copy(y_sb, y_ps)
                else:
                    nc.vector.tensor_copy(y_sb, y_ps)
                ys = y_d[it * 128 : (it + 1) * 128, oc * 512 : (oc + 1) * 512]
                if split_dma:
                    nc.sync.dma_start(out=ys[:, 0:256], in_=y_sb[:, 0:256])
                    nc.sync.dma_start(out=ys[:, 256:512], in_=y_sb[:, 256:512])
                else:
                    nc.sync.dma_start(out=ys, in_=y_sb)

            def emit_outproj_tile(it, scalar_cast=False):
                for oc in range(2):
                    emit_outproj_group(it, oc, scalar_cast=scalar_cast)

            # ---- prologue: projections for chunk 0 ----------------------
            for tt in range(JT_PER_CH):
                emit_v_tile(tt)
            for pr in range(4):
                emit_qk_pair(pr, 0)

            # ---- main pipelined loop ------------------------------------
            for c in range(NCH):
                cs = slice(c * CH, (c + 1) * CH)
                n_j = (c + 1) * JT_PER_CH
                for pr in range(4):
                    av_t = [
                        ps_av.tile([65, 512], F32, name="av", tag="av")
                        for _ in range(2)
                    ]
                    last_pair = c == NCH - 1 and pr == 3

                    def emit_norm_half(h, avs=av_t, pr=pr):
                        # causal: av cols [0:256] take their last write at
                        # J=n_j-3, so the lo half can normalize while the
                        # hi half still accumulates (stop is sim-only).
                        s = slice(h * 256, (h + 1) * 256)
                        zp2 = zpool.tile([128, 256], BF16, name="zp2", tag="z2")
                        lr = [
                            small.tile([1, 256], F32, name="lr2", tag="l2")
                            for _ in range(2)
                        ]
                        for hh in range(2):
                            nc.vector.tensor_copy(lr[hh], avs[hh][64:65, s])
                        for hh in range(2):
                            nc.vector.tensor_copy(
                                zp2[hh * 64 : (hh + 1) * 64, :], avs[hh][0:64, s]
                            )
                        lb2 = lbpool.tile([128, 256], F32, name="lb2", tag="lb2")
                        lbB2 = lbpool.tile([64, 256], F32, name="lbB2", tag="lbB2")
                        nc.gpsimd.partition_broadcast(
                            lb2[0:64, :], lr[0], channels=64
                        )
                        nc.gpsimd.partition_broadcast(lbB2, lr[1], channels=64)
                        nc.vector.tensor_copy(lb2[64:128, :], lbB2)
                        li2 = lbpool.tile([128, 256], F32, name="li2", tag="li2")
                        nc.vector.reciprocal_approx_fast(li2, lb2)
                        cc_t2, cbase2 = concat_of(NCH - 1)
                        nc.vector.tensor_mul(
                            cc_t2[:, pr, cbase2 + h * 256 : cbase2 + (h + 1) * 256],
                            zp2,
                            li2,
                        )

                    s_tiles = {}

                    def emit_S(J, pr=pr, c=c, s_tiles=s_tiles):
                        r = J - c * JT_PER_CH
                        off = max(0, r) * 128
                        w = CH - off
                        spair = ps_s.tile([128, 1024], F32, name="spair", tag="s")
                        # head A at [off, CH); head B packed at [512, 512+w)
                        # so the exp range [off, 512+w) is gap-free.
                        for hh in range(2):
                            hs = slice(hh * 64, (hh + 1) * 64)
                            dst = (
                                spair[:, off:CH]
                                if hh == 0
                                else spair[:, 512 : 512 + w]
                            )
                            nc.tensor.matmul(
                                dst,
                                kT_sb[hs, pr, J * 128 : (J + 1) * 128],
                                qT_sb[hs, pr, c * CH + off : (c + 1) * CH],
                                start=True,
                                stop=True,
                            )
                        s_tiles[J] = (spair, off)

                    emit_S(0)
                    for J in range(n_j):
                        if J + 1 < n_j:
                            emit_S(J + 1)
                        spair, off = s_tiles.pop(J)
                        w = CH - off
                        r = J - c * JT_PER_CH
                        b_sl = [slice(off, CH), slice(512, 512 + w)]
                        e_pair = est_pool.tile([128, 1024], BF16, name="e_t", tag="e")
                        nc.scalar.activation(
                            e_pair[:, off : 512 + w],
                            spair[:, off : 512 + w],
                            Exp,
                            scale=0.125,
                        )
                        if 0 <= r < JT_PER_CH:
                            for hh in range(2):
                                d0 = b_sl[hh].start
                                nc.vector.tensor_mul(
                                    e_pair[:, d0 : d0 + 128],
                                    e_pair[:, d0 : d0 + 128],
                                    tri_sb,
                                )
                        for hh in range(2):
                            h = 2 * pr + hh
                            nc.tensor.matmul(
                                av_t[hh][:, off:CH],
                                v_sb[:, J, h * 65 : (h + 1) * 65],
                                e_pair[:, b_sl[hh]],
                                start=(J == 0),
                                stop=(J == n_j - 1),
                            )
                        if last_pair and J == n_j - 3:
                            emit_norm_half(0)
                            emit_outproj_tile((NCH - 1) * JT_PER_CH)
                            emit_outproj_tile((NCH - 1) * JT_PER_CH + 1)
                    if last_pair:
                        emit_norm_half(1)
                        emit_outproj_tile((NCH - 1) * JT_PER_CH + 2)
                        emit_outproj_tile((NCH - 1) * JT_PER_CH + 3)
                        continue
                    # ---- evacuate + normalize this pair -----------------
                    zpair = zpool.tile([128, 512], BF16, name="zpair", tag="z")
                    lrow = [
                        small.tile([1, 512], F32, name="lrow", tag="l")
                        for _ in range(2)
                    ]
                    # l rows first: the GPSIMD broadcasts (longest part of the
                    # normalization chain) can start while the z casts run
                    for hh in range(2):
                        nc.vector.tensor_copy(lrow[hh], av_t[hh][64:65, :])
                    for hh in range(2):
                        nc.vector.tensor_copy(
                            zpair[hh * 64 : (hh + 1) * 64, :], av_t[hh][0:64, :]
                        )
                    av_t = None
                    # normalization chain part A: broadcasts + reciprocal
                    # (no concat write, so PE work emitted after this picks
                    # up no false dependency on it).  partition_broadcast
                    # writes at base partition 0 only; assemble the
                    # [128,512] tile with a quadrant-shift copy.
                    lbc = lbpool.tile([128, 512], F32, name="lbc", tag="lbc")
                    lbcB = lbpool.tile([64, 512], F32, name="lbcB", tag="lbcB")
                    nc.gpsimd.partition_broadcast(lbc[0:64, :], lrow[0], channels=64)
                    nc.gpsimd.partition_broadcast(lbcB, lrow[1], channels=64)
                    nc.vector.tensor_copy(lbc[64:128, :], lbcB)
                    linv = lbpool.tile([128, 512], F32, name="linv", tag="linv")
                    if RECIP_MODE == "approx":
                        # 51-ULP fast reciprocal is ample for the softmax
                        # denominator (values are O(1..1e3), well-conditioned)
                        nc.vector.reciprocal_approx_fast(linv, lbc)
                    elif RECIP_MODE == "approx2":
                        scratch = lbpool.tile([128, 512], F32, name="lscr", tag="lscr")
                        nc.vector.reciprocal_approx_accurate(linv, lbc, scratch)
                    else:
                        nc.vector.reciprocal(linv, lbc)
                    # PE-side work (projections + outproj) before the concat
                    # mul so it doesn't pick up a conservative dependency on
                    # this pair's concat write.
                    if c + 1 < NCH:
                        emit_qk_pair(pr, c + 1)
                        emit_v_tile((c + 1) * JT_PER_CH + pr)
                        if c > 0:
                            # ScalarE cast: keeps ps_yp recycling off the
                            # DVE norm chain (ACT has slack in chunks 1-2)
                            emit_outproj_tile(
                                (c - 1) * JT_PER_CH + pr, scalar_cast=True
                            )
                    else:
                        # last chunk: hold back two outproj tiles as PE
                        # filler for the final normalization chain (their
                        # reads live in the other parity tile, so no false
                        # dependency on this chunk's concat writes)
                        if pr < 2:
                            emit_outproj_tile((c - 1) * JT_PER_CH + pr)
                        elif pr == 2:
                            emit_outproj_tile((c - 1) * JT_PER_CH + 2)
                            emit_outproj_tile((c - 1) * JT_PER_CH + 3)
                    cc_t, cbase = concat_of(c)
                    nc.vector.tensor_mul(
                        cc_t[:, pr, cbase : cbase + CH], zpair, linv
                    )
            # (epilogue outproj tiles are emitted inside the last pair's
            # block, interleaved with its split normalization)
    nc.finalize()
    return nc


# ---------------------------------------------------------------------------
# Fallback for non-causal masks: the original two-phase kernel.
def build_core_program_general(T=2048, mask_mode="causal", has_bias=False):
    """One-core program; same NEFF runs SPMD on all 8 cores."""
    import os as _os

    SKEW = _os.environ.get("K_SKEW", "1") == "1"
    RECIP_MODE = _os.environ.get("K_RECIP", "exact")
    _apply_tile_patch()
    NT = T // 128            # 128-row t-tiles
    CH = min(512, T)         # i-chunk width
    NCH = T // CH            # chunks
    JT_PER_CH = CH // 128    # j-tiles per chunk

    nc = bacc.Bacc("TRN2", target_bir_lowering=False, debug=False)
    xT_d = nc.declare_dram_parameter("xT", [D, T], BF16, isOutput=False)
    wq_d = nc.declare_dram_parameter("wq", [D, 512], BF16, isOutput=False)
    wk_d = nc.declare_dram_parameter("wk", [D, 512], BF16, isOutput=False)
    wv_d = nc.declare_dram_parameter("wv", [D, 512], BF16, isOutput=False)
    wo_d = nc.declare_dram_parameter("wo", [512, D], BF16, isOutput=False)
    tri_d = nc.declare_dram_parameter("tri", [128, 128], BF16, isOutput=False)
    idn_d = nc.declare_dram_parameter("idn", [64, 64], BF16, isOutput=False)
    if mask_mode == "general":
        mt_d = nc.declare_dram_parameter("maskT", [T, T], BF16, isOutput=False)
    if has_bias:
        wqb_d = nc.declare_dram_parameter("wqb", [1, 512], BF16, isOutput=False)
        wkb_d = nc.declare_dram_parameter("wkb", [1, 512], BF16, isOutput=False)
        wvb_d = nc.declare_dram_parameter("wvb", [1, 512], BF16, isOutput=False)
    y_d = nc.declare_dram_parameter("y", [T, D], F32, isOutput=True)

    Exp = mybir.ActivationFunctionType.Exp

    with tile.TileContext(nc) as tc:
        with (
            tc.tile_pool(name="singles", bufs=1) as singles,
            tc.tile_pool(name="est", bufs=4) as est_pool,
            tc.tile_pool(name="small", bufs=6) as small,
            tc.tile_pool(name="yout", bufs=3) as yout,
            tc.tile_pool(name="ps_big", bufs=2, space="PSUM") as ps_big,
            tc.tile_pool(name="ps_av", bufs=2, space="PSUM") as ps_av,
            tc.tile_pool(name="ps_y", bufs=2, space="PSUM") as ps_y,
        ):
            # ---- loads -------------------------------------------------
            xT_sb = singles.tile([128, KD, T], BF16, name="xT_sb")
            nc.sync.dma_start(
                out=xT_sb, in_=xT_d[:, :].rearrange("(kt p) t -> p kt t", p=128)
            )
            wq_sb = singles.tile([128, KD, 512], BF16, name="wq_sb")
            nc.sync.dma_start(
                out=wq_sb, in_=wq_d[:, :].rearrange("(kt p) n -> p kt n", p=128)
            )
            wk_sb = singles.tile([128, KD, 512], BF16, name="wk_sb")
            nc.sync.dma_start(
                out=wk_sb, in_=wk_d[:, :].rearrange("(kt p) n -> p kt n", p=128)
            )
            wv_sb = singles.tile([128, KD, 512], BF16, name="wv_sb")
            nc.sync.dma_start(
                out=wv_sb, in_=wv_d[:, :].rearrange("(kt p) n -> p kt n", p=128)
            )
            wo_sb = singles.tile([128, 4, D], BF16, name="wo_sb")
            nc.sync.dma_start(
                out=wo_sb, in_=wo_d[:, :].rearrange("(ct p) o -> p ct o", p=128)
            )
            tri_sb = singles.tile([128, 128], BF16, name="tri_sb")
            nc.sync.dma_start(out=tri_sb, in_=tri_d[:, :])
            idn_sb = singles.tile([64, 64], BF16, name="idn_sb")
            nc.sync.dma_start(out=idn_sb, in_=idn_d[:, :])
            if has_bias:
                wqb_sb = singles.tile([1, 512], BF16, name="wqb_sb")
                nc.sync.dma_start(out=wqb_sb, in_=wqb_d[:, :])
                wkb_sb = singles.tile([1, 512], BF16, name="wkb_sb")
                nc.sync.dma_start(out=wkb_sb, in_=wkb_d[:, :])
                wvb_sb = singles.tile([1, 512], BF16, name="wvb_sb")
                nc.sync.dma_start(out=wvb_sb, in_=wvb_d[:, :])
                ones_sb = singles.tile([1, T], BF16, name="ones_sb")
                nc.vector.memset(ones_sb, 1.0)

            # ---- v projection: v_sb [t-tile, 8 heads x (64 v + 1 one)] -
            v_sb = singles.tile([128, NT, 8 * 65], BF16, name="v_sb")
            for tt in range(NT):
                v_ps = ps_big.tile([128, 1024], F32, name="v_ps", tag="big")
                for kt in range(KD):
                    nc.tensor.matmul(
                        v_ps[:, 0:512],
                        xT_sb[:, kt, tt * 128 : (tt + 1) * 128],
                        wv_sb[:, kt, :],
                        start=(kt == 0),
                        stop=(kt == KD - 1 and not has_bias),
                    )
                if has_bias:
                    nc.tensor.matmul(
                        v_ps[:, 0:512],
                        ones_sb[0:1, tt * 128 : (tt + 1) * 128],
                        wvb_sb[0:1, :],
                        start=False,
                        stop=True,
                    )
                v_view = v_sb[:, tt, :].rearrange("p (h x) -> p h x", x=65)
                nc.vector.tensor_copy(
                    v_view[:, :, 0:64],
                    v_ps[:, 0:512].rearrange("p (h x) -> p h x", x=64),
                )
                nc.vector.memset(v_view[:, :, 64:65], 1.0)

            # ---- q/k projections: qT/kT [pair, 128(2 heads x 64e), T] --
            qT_sb = singles.tile([128, 4, T], BF16, name="qT_sb")
            kT_sb = singles.tile([128, 4, T], BF16, name="kT_sb")
            for pr in range(4):
                for c in range(NCH):
                    cs = slice(c * CH, (c + 1) * CH)
                    qk_ps = ps_big.tile([128, 1024], F32, name="qk_ps", tag="big")
                    for kt in range(KD):
                        nc.tensor.matmul(
                            qk_ps[:, 0:CH],
                            wq_sb[:, kt, pr * 128 : (pr + 1) * 128],
                            xT_sb[:, kt, cs],
                            start=(kt == 0),
                            stop=(kt == KD - 1 and not has_bias),
                        )
                    if has_bias:
                        nc.tensor.matmul(
                            qk_ps[:, 0:CH],
                            wqb_sb[0:1, pr * 128 : (pr + 1) * 128],
                            ones_sb[0:1, cs],
                            start=False,
                            stop=True,
                        )
                    for kt in range(KD):
                        nc.tensor.matmul(
                            qk_ps[:, 512 : 512 + CH],
                            wk_sb[:, kt, pr * 128 : (pr + 1) * 128],
                            xT_sb[:, kt, cs],
                            start=(kt == 0),
                            stop=(kt == KD - 1 and not has_bias),
                        )
                    if has_bias:
                        nc.tensor.matmul(
                            qk_ps[:, 512 : 512 + CH],
                            wkb_sb[0:1, pr * 128 : (pr + 1) * 128],
                            ones_sb[0:1, cs],
                            start=False,
                            stop=True,
                        )
                    nc.vector.tensor_copy(qT_sb[:, pr, cs], qk_ps[:, 0:CH])
                    nc.vector.tensor_copy(kT_sb[:, pr, cs], qk_ps[:, 512 : 512 + CH])

            # ---- attention + output projection, chunk by chunk ---------
            concat_sb = singles.tile([128, 4, T], BF16, name="concat_sb")
            if mask_mode == "general":
                _mt_cm = tc.tile_pool(name="mtiles", bufs=NT + 2)
                mt_pool = _mt_cm.__enter__()

            def emit_outproj_tile(it):
                y_sb = yout.tile([128, D], F32, name="y_sb", tag="y")
                for oc in range(2):
                    y_ps = ps_y.tile([128, 512], F32, name="y_ps", tag="y")
                    for ct in range(4):
                        nc.tensor.matmul(
                            y_ps,
                            concat_sb[:, ct, it * 128 : (it + 1) * 128],
                            wo_sb[:, ct, oc * 512 : (oc + 1) * 512],
                            start=(ct == 0),
                            stop=(ct == 3),
                        )
                    nc.vector.tensor_copy(y_sb[:, oc * 512 : (oc + 1) * 512], y_ps)
                nc.sync.dma_start(out=y_d[it * 128 : (it + 1) * 128, :], in_=y_sb)

            for c in range(NCH):
                cs = slice(c * CH, (c + 1) * CH)
                n_j = (c + 1) * JT_PER_CH if mask_mode == "causal" else NT
                if mask_mode == "general":
                    m_tiles = []
                    for J in range(n_j):
                        mt = mt_pool.tile([128, 512], BF16, name="mt", tag="mt")
                        nc.sync.dma_start(
                            out=mt[:, :CH],
                            in_=mt_d[J * 128 : (J + 1) * 128, cs],
                        )
                        m_tiles.append(mt)
                for pr in range(4):
                    av_t = [
                        ps_av.tile([65, 512], F32, name="av", tag="av")
                        for _ in range(2)
                    ]
                    s_tiles = {}

                    def emit_S(J, pr=pr, c=c, s_tiles=s_tiles):
                        r = J - c * JT_PER_CH
                        off = max(0, r) * 128 if mask_mode == "causal" else 0
                        w = CH - off
                        spair = ps_big.tile([128, 1024], F32, name="spair", tag="big")
                        for hh in range(2):
                            hs = slice(hh * 64, (hh + 1) * 64)
                            dst = (
                                spair[:, off:CH]
                                if hh == 0
                                else spair[:, 512 : 512 + w]
                            )
                            nc.tensor.matmul(
                                dst,
                                kT_sb[hs, pr, J * 128 : (J + 1) * 128],
                                qT_sb[hs, pr, c * CH + off : (c + 1) * CH],
                                start=True,
                                stop=True,
                            )
                        s_tiles[J] = (spair, off)

                    if SKEW:
                        emit_S(0)
                    for J in range(n_j):
                        if SKEW:
                            if J + 1 < n_j:
                                emit_S(J + 1)
                        else:
                            emit_S(J)
                        spair, off = s_tiles.pop(J)
                        w = CH - off
                        r = J - c * JT_PER_CH
                        b_sl = [slice(off, CH), slice(512, 512 + w)]
                        e_pair = est_pool.tile([128, 1024], BF16, name="e_t", tag="e")
                        nc.scalar.activation(
                            e_pair[:, off : 512 + w],
                            spair[:, off : 512 + w],
                            Exp,
                            scale=0.125,
                        )
                        if mask_mode == "causal" and 0 <= r < JT_PER_CH:
                            for hh in range(2):
                                d0 = b_sl[hh].start
                                nc.vector.tensor_mul(
                                    e_pair[:, d0 : d0 + 128],
                                    e_pair[:, d0 : d0 + 128],
                                    tri_sb,
                                )
                        elif mask_mode == "general":
                            for hh in range(2):
                                nc.vector.tensor_mul(
                                    e_pair[:, b_sl[hh]],
                                    e_pair[:, b_sl[hh]],
                                    m_tiles[J][:, :CH],
                                )
                        for hh in range(2):
                            h = 2 * pr + hh
                            nc.tensor.matmul(
                                av_t[hh][:, off:CH],
                                v_sb[:, J, h * 65 : (h + 1) * 65],
                                e_pair[:, b_sl[hh]],
                                start=(J == 0),
                                stop=(J == n_j - 1),
                            )
                    for hh in range(2):
                        hs = slice(hh * 64, (hh + 1) * 64)
                        av = av_t[hh]
                        l_sb = small.tile([1, 512], F32, name="l_sb", tag="lsb")
                        nc.vector.tensor_copy(l_sb[:, :CH], av[64:65, :CH])
                        zraw = small.tile([64, 512], BF16, name="zraw", tag="zraw")
                        nc.vector.tensor_copy(zraw[:, :CH], av[0:64, :CH])
                        av = None
                        linv = small.tile([1, 512], F32, name="linv", tag="linv")
                        if RECIP_MODE == "approx":
                            nc.vector.reciprocal_approx_fast(
                                linv[:, :CH], l_sb[:, :CH]
                            )
                        elif RECIP_MODE == "lnexp":
                            lt = small.tile([1, 512], F32, name="lt", tag="lt")
                            nc.scalar.activation(
                                lt[:, :CH],
                                l_sb[:, :CH],
                                mybir.ActivationFunctionType.Ln,
                                scale=1.0,
                            )
                            nc.scalar.activation(
                                linv[:, :CH],
                                lt[:, :CH],
                                Exp,
                                scale=-1.0,
                            )
                        else:
                            nc.vector.reciprocal(linv[:, :CH], l_sb[:, :CH])
                        lbc = small.tile([64, 512], F32, name="lbc", tag="lbc")
                        nc.gpsimd.partition_broadcast(
                            lbc[:, :CH], linv[:, :CH], channels=64
                        )
                        nc.vector.tensor_mul(
                            concat_sb[hs, pr, cs], zraw[:, :CH], lbc[:, :CH]
                        )
                    if c > 0:
                        emit_outproj_tile((c - 1) * JT_PER_CH + pr)
                for it in range((NCH - 1) * JT_PER_CH, NCH * JT_PER_CH):
                    if c == NCH - 1:
                        emit_outproj_tile(it)
            if mask_mode == "general":
                _mt_cm.__exit__(None, None, None)
    nc.finalize()
    return nc


# ---------------------------------------------------------------------------
# Optional NTFF profiling (test.py sets TRACE=True). Registers the missing
# antenv.axon_hooks module so run_bass_kernel_spmd's trace path works.
TRACE = False
LAST_EXEC_TIME_NS = None
LAST_RESULTS = None


def _ensure_ntff_hook():
    import sys as _sys
    import types as _types

    if "antenv.axon_hooks" in _sys.modules:
        return
    mod = _types.ModuleType("antenv.axon_hooks")
    state = {"hook": None}
    mod.set_axon_ntff_profile_hook = lambda h: state.__setitem__("hook", h)
    mod.get_axon_ntff_profile_hook = lambda: state["hook"]
    _sys.modules["antenv.axon_hooks"] = mod
    import antenv

    antenv.axon_hooks = mod
    try:
        from trn_agent_boot.trn_boot import _ntff_profile_via_ctypes

        hook = _ntff_profile_via_ctypes("/opt/axon/libaxon_pjrt.so")
        if hook is not None:
            mod.set_axon_ntff_profile_hook(hook)
    except Exception:
        pass


_PROGRAM_CACHE = {}


def _get_program(T, mask_mode, has_bias):
    key = (T, mask_mode, has_bias)
    if key not in _PROGRAM_CACHE:
        if mask_mode == "causal":
            _PROGRAM_CACHE[key] = build_core_program(T, has_bias)
        else:
            _PROGRAM_CACHE[key] = build_core_program_general(T, mask_mode, has_bias)
    return _PROGRAM_CACHE[key]


def _mask_mode_of(mask):
    m = np.asarray(mask)
    if m.all():
        return "full"
    T = m.shape[0]
    tril = np.tril(np.ones((T, T), dtype=bool))
    if np.array_equal(m.astype(bool), tril):
        return "causal"
    return "general"


def kernel(x, mask, Wq, bq, Wk, bk, Wv, bv, Wo, bo):
    x = np.asarray(x)
    B, T, D_ = x.shape
    H = Wq.shape[0]
    assert D_ == D and H == 16
    mask_mode = _mask_mode_of(mask)
    has_bias = bool(
        np.any(np.asarray(bq)) or np.any(np.asarray(bk)) or np.any(np.asarray(bv))
    )
    nc = _get_program(T, mask_mode, has_bias)

    tri = np.triu(np.ones((128, 128), dtype=np.float32)).astype(nbf16)
    idn = np.eye(64, dtype=np.float32).astype(nbf16)
    if mask_mode == "general":
        maskT = np.ascontiguousarray(np.asarray(mask).T.astype(np.float32)).astype(
            nbf16
        )

    in_maps = []
    for core in range(8):
        b, g = core // 2, core % 2
        hsl = slice(g * HL, (g + 1) * HL)
        # (h, d, e) -> (d, h*e)
        wq = np.ascontiguousarray(
            np.transpose(np.asarray(Wq)[hsl], (1, 0, 2)).reshape(D, 512)
        ).astype(nbf16)
        wk = np.ascontiguousarray(
            np.transpose(np.asarray(Wk)[hsl], (1, 0, 2)).reshape(D, 512)
        ).astype(nbf16)
        wv = np.ascontiguousarray(
            np.transpose(np.asarray(Wv)[hsl], (1, 0, 2)).reshape(D, 512)
        ).astype(nbf16)
        wo = np.ascontiguousarray(np.asarray(Wo)[:, g * 512 : (g + 1) * 512].T).astype(
            nbf16
        )
        im = {
            "xT": np.ascontiguousarray(x[b].T).astype(nbf16),
            "wq": wq,
            "wk": wk,
            "wv": wv,
            "wo": wo,
            "tri": tri,
        }
        if mask_mode != "causal":
            im["idn"] = idn
        if mask_mode == "general":
            im["maskT"] = maskT
        if has_bias:
            im["wqb"] = np.asarray(bq)[hsl].reshape(1, 512).astype(nbf16)
            im["wkb"] = np.asarray(bk)[hsl].reshape(1, 512).astype(nbf16)
            im["wvb"] = np.asarray(bv)[hsl].reshape(1, 512).astype(nbf16)
        in_maps.append(im)

    global LAST_EXEC_TIME_NS, LAST_RESULTS
    if TRACE:
        _ensure_ntff_hook()
    res = run_bass_kernel_spmd(nc, in_maps, core_ids=list(range(8)), trace=TRACE)
    LAST_RESULTS = res
    if TRACE:
        LAST_EXEC_TIME_NS = res.exec_time_ns
    out = np.empty((B, T, D), dtype=np.float32)
    bo_f = np.asarray(bo, dtype=np.float32)
    for b in range(B):
        out[b] = (
            np.asarray(res.results[2 * b]["y"], dtype=np.float32)
            + np.asarray(res.results[2 * b + 1]["y"], dtype=np.float32)
            + bo_f
        )
    return out


# revision 47
# speedup vs baseline: 1.0329x; 1.0033x over previous
"""Multi-head causal attention (B=4, T=2048, D=1024, H=16, DH=64) on 8 trn2 cores.

Sharding: core = 2*b + g  (b = batch 0..3, g = head-group 0..1, 8 heads each).
Each core computes q/k/v projections for its 8 heads, causal attention, and the
row-parallel slice of the output projection; the host sums the two partial
outputs per batch and adds the output bias.

v2: single software-pipelined loop — the q/k/v projections for chunk c+1 and
the output projection for chunk c-1 are interleaved (in PE issue order) with
the attention j-loop of chunk c, so ScalarE's exp stream and the DVE
normalization work overlap the projection matmuls instead of running in a
separate phase (429us -> ~285us).  Softmax normalization is per (chunk,
pair): l rows are broadcast first (GPSIMD, base-partition-0 only — writing
at partition offset 64 silently corrupts, hence the quadrant-shift copy),
then inverted with the fast approx reciprocal on all 128 lanes, replacing
the 3.3us single-lane exact reciprocals.  q/k/v PSUM evacuations run on
ScalarE to keep the DVE queue short.  Output is stored bf16 (host sums
partials in fp32).

Per-core dataflow (all matmuls bf16 -> fp32 PSUM):
  xT (D,T) stationary-side input, host pre-transposed, DMA'd in 4 col-chunks
  qT/kT  [2-head pairs, 128 x T]  = Wpair.T @ x.T      (PE, K=128 d-tiles)
  v      [T-tiles 128 x 520]      = x @ Wv (+ ones col per head for row sums)
  ST     [j-tile 128, i-chunk 512] = kT.T @ qT          (K=64, 2 heads packed
                                                         in row groups 0-1/2-3)
  expST  = exp(ST/8)  (ScalarE, scale fused; causal: upper tiles trimmed,
                       diagonal tiles masked with a host 0/1 triangle)
  av     [65, 512] += v_aug.T @ expST  (row 64 = softmax denominator l)
  z      = av[0:64] * (1/l)  (GPSIMD partition_broadcast of l, then 2-ULP
                              reciprocal + multiply on DVE, 128 lanes)
  y      [T x 1024] = concatT.T @ WoT_g slices (K=128 c-tiles, bf16 out)
"""

import numpy as np
import ml_dtypes

import concourse.bass as bass
import concourse.bacc as bacc
import concourse.mybir as mybir
import concourse.tile as tile
from concourse.vector_clock import ScopedClock
from concourse.bass_utils import run_bass_kernel_spmd

BF16 = mybir.dt.bfloat16
F32 = mybir.dt.float32
nbf16 = ml_dtypes.bfloat16

D = 1024
DH = 64
HL = 8          # heads per core
KD = D // 128   # d-tiles


# ---------------------------------------------------------------------------
# Walrus in this build rejects >1 sync-wait on SP TPB_CTRL instructions; split
# the TileContext tail-drain's sem waits into single-wait SP nops.
def _patched_drain_and_barrier(self, tick_clock, wait_clock):
    nc = self.nc
    collector = nc.sync.nop()
    wait_clock.add_sem_waits(
        collector.ins, ScopedClock({None: tick_clock.global_clock})
    )
    si = collector.ins.sync_info
    waits = list(si.on_wait) if si and si.on_wait else []
    if si is not None:
        si.on_wait = waits[:1]
    for w in waits[1:]:
        extra = nc.sync.nop()
        esi = extra.ins.sync_info
        if esi is None:
            extra.ins.sync_info = mybir.SyncInfo(on_wait=[w], on_update=[])
        else:
            esi.on_wait = [w]
    nc.sync.drain()
    nc.all_engine_barrier()
    popped = nc._tile_sem_poison_stack.pop()
    assert popped is self._sem_poison
    nc.clear_and_free_semaphores(list(self.sems.allocated().values()))
    nc.all_engine_barrier()


def _apply_tile_patch():
    tile.TileContext._drain_and_barrier = _patched_drain_and_barrier


# ---------------------------------------------------------------------------
def build_core_program(T=2048, has_bias=False):
    """Causal fast path: one-core program; same NEFF runs SPMD on all 8 cores."""
    import os as _os

    RECIP_MODE = _os.environ.get("K_RECIP", "approx")
    _apply_tile_patch()
    NT = T // 128            # 128-row t-tiles
    CH = min(512, T)         # i-chunk width
    NCH = T // CH            # chunks
    JT_PER_CH = CH // 128    # j-tiles per chunk

    nc = bacc.Bacc("TRN2", target_bir_lowering=False, debug=False)
    xT_d = nc.declare_dram_parameter("xT", [D, T], BF16, isOutput=False)
    wq_d = nc.declare_dram_parameter("wq", [D, 512], BF16, isOutput=False)
    wk_d = nc.declare_dram_parameter("wk", [D, 512], BF16, isOutput=False)
    wv_d = nc.declare_dram_parameter("wv", [D, 512], BF16, isOutput=False)
    wo_d = nc.declare_dram_parameter("wo", [512, D], BF16, isOutput=False)
    tri_d = nc.declare_dram_parameter("tri", [128, 128], BF16, isOutput=False)
    if has_bias:
        wqb_d = nc.declare_dram_parameter("wqb", [1, 512], BF16, isOutput=False)
        wkb_d = nc.declare_dram_parameter("wkb", [1, 512], BF16, isOutput=False)
        wvb_d = nc.declare_dram_parameter("wvb", [1, 512], BF16, isOutput=False)
    y_d = nc.declare_dram_parameter("y", [T, D], BF16, isOutput=True)

    Exp = mybir.ActivationFunctionType.Exp

    with tile.TileContext(nc) as tc:
        with (
            tc.tile_pool(name="singles", bufs=1) as singles,
            tc.tile_pool(name="est", bufs=6) as est_pool,
            tc.tile_pool(name="zp", bufs=8) as zpool,
            tc.tile_pool(name="small", bufs=6) as small,
            tc.tile_pool(name="lb", bufs=3) as lbpool,
            tc.tile_pool(name="yout", bufs=6) as yout,
            tc.tile_pool(name="ps_s", bufs=2, space="PSUM") as ps_s,
            tc.tile_pool(name="ps_av", bufs=2, space="PSUM") as ps_av,
            tc.tile_pool(name="ps_yp", bufs=2, space="PSUM") as ps_yp,
        ):
            # ---- loads (ordered so chunk-0 work can start early) ---------
            tri_sb = singles.tile([128, 128], BF16, name="tri_sb")
            nc.sync.dma_start(out=tri_sb, in_=tri_d[:, :])
            xT_sb = singles.tile([128, KD, T], BF16, name="xT_sb")
            wv_sb = singles.tile([128, KD, 512], BF16, name="wv_sb")
            for q in range(4):
                ks = slice(q * (KD // 4), (q + 1) * (KD // 4))
                ds = slice(q * (D // 4), (q + 1) * (D // 4))
                nc.sync.dma_start(
                    out=xT_sb[:, ks, 0:128],
                    in_=xT_d[ds, 0:128].rearrange("(kt p) t -> p kt t", p=128),
                )
                nc.sync.dma_start(
                    out=wv_sb[:, ks, :],
                    in_=wv_d[ds, :].rearrange("(kt p) n -> p kt n", p=128),
                )
            nc.sync.dma_start(
                out=xT_sb[:, :, 128:CH],
                in_=xT_d[:, 128:CH].rearrange("(kt p) t -> p kt t", p=128),
            )
            wq_sb = singles.tile([128, KD, 512], BF16, name="wq_sb")
            nc.sync.dma_start(
                out=wq_sb, in_=wq_d[:, :].rearrange("(kt p) n -> p kt n", p=128)
            )
            wk_sb = singles.tile([128, KD, 512], BF16, name="wk_sb")
            nc.sync.dma_start(
                out=wk_sb, in_=wk_d[:, :].rearrange("(kt p) n -> p kt n", p=128)
            )
            for cc in range(1, NCH):
                nc.sync.dma_start(
                    out=xT_sb[:, :, cc * CH : (cc + 1) * CH],
                    in_=xT_d[:, cc * CH : (cc + 1) * CH].rearrange(
                        "(kt p) t -> p kt t", p=128
                    ),
                )
            wo_sb = singles.tile([128, 4, D], BF16, name="wo_sb")
            nc.sync.dma_start(
                out=wo_sb, in_=wo_d[:, :].rearrange("(ct p) o -> p ct o", p=128)
            )
            if has_bias:
                wqb_sb = singles.tile([1, 512], BF16, name="wqb_sb")
                nc.sync.dma_start(out=wqb_sb, in_=wqb_d[:, :])
                wkb_sb = singles.tile([1, 512], BF16, name="wkb_sb")
                nc.sync.dma_start(out=wkb_sb, in_=wkb_d[:, :])
                wvb_sb = singles.tile([1, 512], BF16, name="wvb_sb")
                nc.sync.dma_start(out=wvb_sb, in_=wvb_d[:, :])
            ones_sb = singles.tile([1, T], BF16, name="ones_sb")
            nc.vector.memset(ones_sb, 1.0)

            # warm the exp activation table while the inputs stream in
            warm_sb = singles.tile([1, 8], F32, name="warm_sb")
            nc.vector.memset(warm_sb, 0.0)
            nc.scalar.activation(warm_sb, warm_sb, Exp, scale=1.0)

            # warm the PE HAM clock gate (idle default is 1.2 GHz; ~3.4us of
            # sustained matmul activity unlocks 2.4 GHz) with throwaway
            # matmuls during the otherwise-idle input-DMA window, so the
            # first real matmuls run at full clock
            ham_sb = singles.tile([128, 128], BF16, name="ham_sb")
            nc.vector.memset(ham_sb, 0.0)
            ham_ps = ps_yp.tile([128, 512], F32, name="ham_ps", tag="yp")
            for _ in range(9):
                nc.tensor.matmul(
                    ham_ps[:, 0:128], ham_sb, ham_sb, start=True, stop=True
                )
                nc.tensor.matmul(
                    ham_ps[:, 128:256], ham_sb, ham_sb, start=True, stop=True
                )
                nc.tensor.matmul(
                    ham_ps[:, 256:384], ham_sb, ham_sb, start=True, stop=True
                )
                nc.tensor.matmul(
                    ham_ps[:, 384:512], ham_sb, ham_sb, start=True, stop=True
                )

            v_sb = singles.tile([128, NT, 8 * 65], BF16, name="v_sb")
            qT_sb = singles.tile([128, 4, T], BF16, name="qT_sb")
            kT_sb = singles.tile([128, 4, T], BF16, name="kT_sb")
            # concat is split by chunk parity so late-chunk normalization
            # writes and earlier-chunk outproj reads live in different tiles
            # (Tile's conservative emission-order dependency tracking would
            # otherwise chain readers to the newest write).
            HT = ((NCH + 1) // 2) * CH
            concat_par = [
                singles.tile([128, 4, HT], BF16, name=f"concat{p}")
                for p in range(2)
            ]

            def concat_of(c):
                return concat_par[c % 2], (c // 2) * CH

            # ---- emission helpers ---------------------------------------
            def emit_v_tile(tt):
                v_ps = ps_yp.tile([128, 512], F32, name="v_ps", tag="yp")
                for kt in range(KD):
                    nc.tensor.matmul(
                        v_ps,
                        xT_sb[:, kt, tt * 128 : (tt + 1) * 128],
                        wv_sb[:, kt, :],
                        start=(kt == 0),
                        stop=(kt == KD - 1 and not has_bias),
                    )
                if has_bias:
                    nc.tensor.matmul(
                        v_ps,
                        ones_sb[0:1, tt * 128 : (tt + 1) * 128],
                        wvb_sb[0:1, :],
                        start=False,
                        stop=True,
                    )
                v_view = v_sb[:, tt, :].rearrange("p (h x) -> p h x", x=65)
                # ScalarE evacuation keeps the DVE queue short so PSUM
                # buffers recycle fast (ACT has slack outside the last chunk)
                nc.scalar.copy(
                    v_view[:, :, 0:64],
                    v_ps.rearrange("p (h x) -> p h x", x=64),
                )
                nc.vector.memset(v_view[:, :, 64:65], 1.0)

            def emit_qk_pair(pr, c):
                cs = slice(c * CH, (c + 1) * CH)
                for which, w_sb, wb_sb, dst in (
                    ("q", wq_sb, wqb_sb if has_bias else None, qT_sb),
                    ("k", wk_sb, wkb_sb if has_bias else None, kT_sb),
                ):
                    qk_ps = ps_yp.tile([128, 512], F32, name="qk_ps", tag="yp")
                    for kt in range(KD):
                        nc.tensor.matmul(
                            qk_ps[:, 0:CH],
                            w_sb[:, kt, pr * 128 : (pr + 1) * 128],
                            xT_sb[:, kt, cs],
                            start=(kt == 0),
                            stop=(kt == KD - 1 and not has_bias),
                        )
                    if has_bias:
                        nc.tensor.matmul(
                            qk_ps[:, 0:CH],
                            wb_sb[0:1, pr * 128 : (pr + 1) * 128],
                            ones_sb[0:1, cs],
                            start=False,
                            stop=True,
                        )
                    nc.scalar.copy(dst[:, pr, cs], qk_ps[:, 0:CH])

            def emit_outproj_group(it, oc, scalar_cast=False, split_dma=False):
                cc_t, cbase = concat_of(it // JT_PER_CH)
                k = it % JT_PER_CH
                y_ps = ps_yp.tile([128, 512], F32, name="y_ps", tag="yp")
                for ct in range(4):
                    nc.tensor.matmul(
                        y_ps,
                        cc_t[:, ct, cbase + k * 128 : cbase + (k + 1) * 128],
                        wo_sb[:, ct, oc * 512 : (oc + 1) * 512],
                        start=(ct == 0),
                        stop=(ct == 3),
                    )
                y_sb = yout.tile([128, 512], BF16, name="y_sb", tag="y")
                if scalar_cast:
                    nc.scalar.copy(y_sb, y_ps)
                else:
                    nc.vector.tensor_copy(y_sb, y_ps)
                ys = y_d[it * 128 : (it + 1) * 128, oc * 512 : (oc + 1) * 512]
                if split_dma:
                    nc.sync.dma_start(out=ys[:, 0:256], in_=y_sb[:, 0:256])
                    nc.sync.dma_start(out=ys[:, 256:512], in_=y_sb[:, 256:512])
                else:
                    nc.sync.dma_start(out=ys, in_=y_sb)

            def emit_outproj_tile(it, scalar_cast=False):
                for oc in range(2):
                    emit_outproj_group(it, oc, scalar_cast=scalar_cast)

            # ---- prologue: projections for chunk 0 ----------------------
            for tt in range(JT_PER_CH):
                emit_v_tile(tt)
            for pr in range(4):
                emit_qk_pair(pr, 0)

            # ---- main pipelined loop ------------------------------------
            for c in range(NCH):
                cs = slice(c * CH, (c + 1) * CH)
                n_j = (c + 1) * JT_PER_CH
                for pr in range(4):
                    av_t = [
                        ps_av.tile([65, 512], F32, name="av", tag="av")
                        for _ in range(2)
                    ]
                    last_pair = c == NCH - 1 and pr == 3

                    def emit_norm_half(h, avs=av_t, pr=pr):
                        # causal: av cols [0:256] take their last write at
                        # J=n_j-3, so the lo half can normalize while the
                        # hi half still accumulates (stop is sim-only).
                        s = slice(h * 256, (h + 1) * 256)
                        zp2 = zpool.tile([128, 256], BF16, name="zp2", tag="z2")
                        lr = [
                            small.tile([1, 256], F32, name="lr2", tag="l2")
                            for _ in range(2)
                        ]
                        for hh in range(2):
                            nc.vector.tensor_copy(lr[hh], avs[hh][64:65, s])
                        for hh in range(2):
                            nc.vector.tensor_copy(
                                zp2[hh * 64 : (hh + 1) * 64, :], avs[hh][0:64, s]
                            )
                        lb2 = lbpool.tile([128, 256], F32, name="lb2", tag="lb2")
                        lbB2 = lbpool.tile([64, 256], F32, name="lbB2", tag="lbB2")
                        nc.gpsimd.partition_broadcast(
                            lb2[0:64, :], lr[0], channels=64
                        )
                        nc.gpsimd.partition_broadcast(lbB2, lr[1], channels=64)
                        nc.vector.tensor_copy(lb2[64:128, :], lbB2)
                        li2 = lbpool.tile([128, 256], F32, name="li2", tag="li2")
                        nc.vector.reciprocal_approx_fast(li2, lb2)
                        cc_t2, cbase2 = concat_of(NCH - 1)
                        nc.vector.tensor_mul(
                            cc_t2[:, pr, cbase2 + h * 256 : cbase2 + (h + 1) * 256],
                            zp2,
                            li2,
                        )

                    s_tiles = {}

                    def emit_S(J, pr=pr, c=c, s_tiles=s_tiles):
                        r = J - c * JT_PER_CH
                        off = max(0, r) * 128
                        w = CH - off
                        spair = ps_s.tile([128, 1024], F32, name="spair", tag="s")
                        # head A at [off, CH); head B packed at [512, 512+w)
                        # so the exp range [off, 512+w) is gap-free.
                        for hh in range(2):
                            hs = slice(hh * 64, (hh + 1) * 64)
                            dst = (
                                spair[:, off:CH]
                                if hh == 0
                                else spair[:, 512 : 512 + w]
                            )
                            nc.tensor.matmul(
                                dst,
                                kT_sb[hs, pr, J * 128 : (J + 1) * 128],
                                qT_sb[hs, pr, c * CH + off : (c + 1) * CH],
                                start=True,
                                stop=True,
                            )
                        s_tiles[J] = (spair, off)

                    emit_S(0)
                    for J in range(n_j):
                        if J + 1 < n_j:
                            emit_S(J + 1)
                        spair, off = s_tiles.pop(J)
                        w = CH - off
                        r = J - c * JT_PER_CH
                        b_sl = [slice(off, CH), slice(512, 512 + w)]
                        e_pair = est_pool.tile([128, 1024], BF16, name="e_t", tag="e")
                        nc.scalar.activation(
                            e_pair[:, off : 512 + w],
                            spair[:, off : 512 + w],
                            Exp,
                            scale=0.125,
                        )
                        if 0 <= r < JT_PER_CH:
                            for hh in range(2):
                                d0 = b_sl[hh].start
                                nc.vector.tensor_mul(
                                    e_pair[:, d0 : d0 + 128],
                                    e_pair[:, d0 : d0 + 128],
                                    tri_sb,
                                )
                        for hh in range(2):
                            h = 2 * pr + hh
                            nc.tensor.matmul(
                                av_t[hh][:, off:CH],
                                v_sb[:, J, h * 65 : (h + 1) * 65],
                                e_pair[:, b_sl[hh]],
                                start=(J == 0),
                                stop=(J == n_j - 1),
                            )
                        if last_pair and J == n_j - 3:
                            emit_norm_half(0)
                            # ScalarE casts: keep the DVE free for the hi
                            # half's normalization chain
                            for k2 in (0, 1):
                                for oc2 in range(2):
                                    emit_outproj_group(
                                        (NCH - 1) * JT_PER_CH + k2,
                                        oc2,
                                        scalar_cast=True,
                                    )
                    if last_pair:
                        emit_norm_half(1)
                        for k2 in (2, 3):
                            for oc2 in range(2):
                                emit_outproj_group(
                                    (NCH - 1) * JT_PER_CH + k2,
                                    oc2,
                                    scalar_cast=(oc2 == 0),
                                    split_dma=(k2 == 3 and oc2 == 1),
                                )
                        continue
                    # ---- evacuate + normalize this pair -----------------
                    zpair = zpool.tile([128, 512], BF16, name="zpair", tag="z")
                    lrow = [
                        small.tile([1, 512], F32, name="lrow", tag="l")
                        for _ in range(2)
                    ]
                    # l rows first: the GPSIMD broadcasts (longest part of the
                    # normalization chain) can start while the z casts run
                    for hh in range(2):
                        nc.vector.tensor_copy(lrow[hh], av_t[hh][64:65, :])
                    for hh in range(2):
                        nc.vector.tensor_copy(
                            zpair[hh * 64 : (hh + 1) * 64, :], av_t[hh][0:64, :]
                        )
                    av_t = None
                    # normalization chain part A: broadcasts + reciprocal
                    # (no concat write, so PE work emitted after this picks
                    # up no false dependency on it).  partition_broadcast
                    # writes at base partition 0 only; assemble the
                    # [128,512] tile with a quadrant-shift copy.
                    lbc = lbpool.tile([128, 512], F32, name="lbc", tag="lbc")
                    lbcB = lbpool.tile([64, 512], F32, name="lbcB", tag="lbcB")
                    nc.gpsimd.partition_broadcast(lbc[0:64, :], lrow[0], channels=64)
                    nc.gpsimd.partition_broadcast(lbcB, lrow[1], channels=64)
                    nc.vector.tensor_copy(lbc[64:128, :], lbcB)
                    linv = lbpool.tile([128, 512], F32, name="linv", tag="linv")
                    if RECIP_MODE == "approx":
                        # 51-ULP fast reciprocal is ample for the softmax
                        # denominator (values are O(1..1e3), well-conditioned)
                        nc.vector.reciprocal_approx_fast(linv, lbc)
                    elif RECIP_MODE == "approx2":
                        scratch = lbpool.tile([128, 512], F32, name="lscr", tag="lscr")
                        nc.vector.reciprocal_approx_accurate(linv, lbc, scratch)
                    else:
                        nc.vector.reciprocal(linv, lbc)
                    # PE-side work (projections + outproj) before the concat
                    # mul so it doesn't pick up a conservative dependency on
                    # this pair's concat write.
                    if c + 1 < NCH:
                        emit_qk_pair(pr, c + 1)
                        emit_v_tile((c + 1) * JT_PER_CH + pr)
                        if c > 0:
                            # ScalarE cast: keeps ps_yp recycling off the
                            # DVE norm chain (ACT has slack in chunks 1-2)
                            emit_outproj_tile(
                                (c - 1) * JT_PER_CH + pr, scalar_cast=True
                            )
                    else:
                        # last chunk: hold back two outproj tiles as PE
                        # filler for the final normalization chain (their
                        # reads live in the other parity tile, so no false
                        # dependency on this chunk's concat writes)
                        if pr < 2:
                            emit_outproj_tile((c - 1) * JT_PER_CH + pr)
                        elif pr == 2:
                            emit_outproj_tile((c - 1) * JT_PER_CH + 2)
                            emit_outproj_tile((c - 1) * JT_PER_CH + 3)
                    cc_t, cbase = concat_of(c)
                    nc.vector.tensor_mul(
                        cc_t[:, pr, cbase : cbase + CH], zpair, linv
                    )
            # (epilogue outproj tiles are emitted inside the last pair's
            # block, interleaved with its split normalization)
    nc.finalize()
    return nc


# ---------------------------------------------------------------------------
# Fallback for non-causal masks: the original two-phase kernel.
def build_core_program_general(T=2048, mask_mode="causal", has_bias=False):
    """One-core program; same NEFF runs SPMD on all 8 cores."""
    import os as _os

    SKEW = _os.environ.get("K_SKEW", "1") == "1"
    RECIP_MODE = _os.environ.get("K_RECIP", "exact")
    _apply_tile_patch()
    NT = T // 128            # 128-row t-tiles
    CH = min(512, T)         # i-chunk width
    NCH = T // CH            # chunks
    JT_PER_CH = CH // 128    # j-tiles per chunk

    nc = bacc.Bacc("TRN2", target_bir_lowering=False, debug=False)
    xT_d = nc.declare_dram_parameter("xT", [D, T], BF16, isOutput=False)
    wq_d = nc.declare_dram_parameter("wq", [D, 512], BF16, isOutput=False)
    wk_d = nc.declare_dram_parameter("wk", [D, 512], BF16, isOutput=False)
    wv_d = nc.declare_dram_parameter("wv", [D, 512], BF16, isOutput=False)
    wo_d = nc.declare_dram_parameter("wo", [512, D], BF16, isOutput=False)
    tri_d = nc.declare_dram_parameter("tri", [128, 128], BF16, isOutput=False)
    idn_d = nc.declare_dram_parameter("idn", [64, 64], BF16, isOutput=False)
    if mask_mode == "general":
        mt_d = nc.declare_dram_parameter("maskT", [T, T], BF16, isOutput=False)
    if has_bias:
        wqb_d = nc.declare_dram_parameter("wqb", [1, 512], BF16, isOutput=False)
        wkb_d = nc.declare_dram_parameter("wkb", [1, 512], BF16, isOutput=False)
        wvb_d = nc.declare_dram_parameter("wvb", [1, 512], BF16, isOutput=False)
    y_d = nc.declare_dram_parameter("y", [T, D], F32, isOutput=True)

    Exp = mybir.ActivationFunctionType.Exp

    with tile.TileContext(nc) as tc:
        with (
            tc.tile_pool(name="singles", bufs=1) as singles,
            tc.tile_pool(name="est", bufs=4) as est_pool,
            tc.tile_pool(name="small", bufs=6) as small,
            tc.tile_pool(name="yout", bufs=3) as yout,
            tc.tile_pool(name="ps_big", bufs=2, space="PSUM") as ps_big,
            tc.tile_pool(name="ps_av", bufs=2, space="PSUM") as ps_av,
            tc.tile_pool(name="ps_y", bufs=2, space="PSUM") as ps_y,
        ):
            # ---- loads -------------------------------------------------
            xT_sb = singles.tile([128, KD, T], BF16, name="xT_sb")
            nc.sync.dma_start(
                out=xT_sb, in_=xT_d[:, :].rearrange("(kt p) t -> p kt t", p=128)
            )
            wq_sb = singles.tile([128, KD, 512], BF16, name="wq_sb")
            nc.sync.dma_start(
                out=wq_sb, in_=wq_d[:, :].rearrange("(kt p) n -> p kt n", p=128)
            )
            wk_sb = singles.tile([128, KD, 512], BF16, name="wk_sb")
            nc.sync.dma_start(
                out=wk_sb, in_=wk_d[:, :].rearrange("(kt p) n -> p kt n", p=128)
            )
            wv_sb = singles.tile([128, KD, 512], BF16, name="wv_sb")
            nc.sync.dma_start(
                out=wv_sb, in_=wv_d[:, :].rearrange("(kt p) n -> p kt n", p=128)
            )
            wo_sb = singles.tile([128, 4, D], BF16, name="wo_sb")
            nc.sync.dma_start(
                out=wo_sb, in_=wo_d[:, :].rearrange("(ct p) o -> p ct o", p=128)
            )
            tri_sb = singles.tile([128, 128], BF16, name="tri_sb")
            nc.sync.dma_start(out=tri_sb, in_=tri_d[:, :])
            idn_sb = singles.tile([64, 64], BF16, name="idn_sb")
            nc.sync.dma_start(out=idn_sb, in_=idn_d[:, :])
            if has_bias:
                wqb_sb = singles.tile([1, 512], BF16, name="wqb_sb")
                nc.sync.dma_start(out=wqb_sb, in_=wqb_d[:, :])
                wkb_sb = singles.tile([1, 512], BF16, name="wkb_sb")
                nc.sync.dma_start(out=wkb_sb, in_=wkb_d[:, :])
                wvb_sb = singles.tile([1, 512], BF16, name="wvb_sb")
                nc.sync.dma_start(out=wvb_sb, in_=wvb_d[:, :])
                ones_sb = singles.tile([1, T], BF16, name="ones_sb")
                nc.vector.memset(ones_sb, 1.0)

            # ---- v projection: v_sb [t-tile, 8 heads x (64 v + 1 one)] -
            v_sb = singles.tile([128, NT, 8 * 65], BF16, name="v_sb")
            for tt in range(NT):
                v_ps = ps_big.tile([128, 1024], F32, name="v_ps", tag="big")
                for kt in range(KD):
                    nc.tensor.matmul(
                        v_ps[:, 0:512],
                        xT_sb[:, kt, tt * 128 : (tt + 1) * 128],
                        wv_sb[:, kt, :],
                        start=(kt == 0),
                        stop=(kt == KD - 1 and not has_bias),
                    )
                if has_bias:
                    nc.tensor.matmul(
                        v_ps[:, 0:512],
                        ones_sb[0:1, tt * 128 : (tt + 1) * 128],
                        wvb_sb[0:1, :],
                        start=False,
                        stop=True,
                    )
                v_view = v_sb[:, tt, :].rearrange("p (h x) -> p h x", x=65)
                nc.vector.tensor_copy(
                    v_view[:, :, 0:64],
                    v_ps[:, 0:512].rearrange("p (h x) -> p h x", x=64),
                )
                nc.vector.memset(v_view[:, :, 64:65], 1.0)

            # ---- q/k projections: qT/kT [pair, 128(2 heads x 64e), T] --
            qT_sb = singles.tile([128, 4, T], BF16, name="qT_sb")
            kT_sb = singles.tile([128, 4, T], BF16, name="kT_sb")
            for pr in range(4):
                for c in range(NCH):
                    cs = slice(c * CH, (c + 1) * CH)
                    qk_ps = ps_big.tile([128, 1024], F32, name="qk_ps", tag="big")
                    for kt in range(KD):
                        nc.tensor.matmul(
                            qk_ps[:, 0:CH],
                            wq_sb[:, kt, pr * 128 : (pr + 1) * 128],
                            xT_sb[:, kt, cs],
                            start=(kt == 0),
                            stop=(kt == KD - 1 and not has_bias),
                        )
                    if has_bias:
                        nc.tensor.matmul(
                            qk_ps[:, 0:CH],
                            wqb_sb[0:1, pr * 128 : (pr + 1) * 128],
                            ones_sb[0:1, cs],
                            start=False,
                            stop=True,
                        )
                    for kt in range(KD):
                        nc.tensor.matmul(
                            qk_ps[:, 512 : 512 + CH],
                            wk_sb[:, kt, pr * 128 : (pr + 1) * 128],
                            xT_sb[:, kt, cs],
                            start=(kt == 0),
                            stop=(kt == KD - 1 and not has_bias),
                        )
                    if has_bias:
                        nc.tensor.matmul(
                            qk_ps[:, 512 : 512 + CH],
                            wkb_sb[0:1, pr * 128 : (pr + 1) * 128],
                            ones_sb[0:1, cs],
                            start=False,
                            stop=True,
                        )
                    nc.vector.tensor_copy(qT_sb[:, pr, cs], qk_ps[:, 0:CH])
                    nc.vector.tensor_copy(kT_sb[:, pr, cs], qk_ps[:, 512 : 512 + CH])

            # ---- attention + output projection, chunk by chunk ---------
            concat_sb = singles.tile([128, 4, T], BF16, name="concat_sb")
            if mask_mode == "general":
                _mt_cm = tc.tile_pool(name="mtiles", bufs=NT + 2)
                mt_pool = _mt_cm.__enter__()

            def emit_outproj_tile(it):
                y_sb = yout.tile([128, D], F32, name="y_sb", tag="y")
                for oc in range(2):
                    y_ps = ps_y.tile([128, 512], F32, name="y_ps", tag="y")
                    for ct in range(4):
                        nc.tensor.matmul(
                            y_ps,
                            concat_sb[:, ct, it * 128 : (it + 1) * 128],
                            wo_sb[:, ct, oc * 512 : (oc + 1) * 512],
                            start=(ct == 0),
                            stop=(ct == 3),
                        )
                    nc.vector.tensor_copy(y_sb[:, oc * 512 : (oc + 1) * 512], y_ps)
                nc.sync.dma_start(out=y_d[it * 128 : (it + 1) * 128, :], in_=y_sb)

            for c in range(NCH):
                cs = slice(c * CH, (c + 1) * CH)
                n_j = (c + 1) * JT_PER_CH if mask_mode == "causal" else NT
                if mask_mode == "general":
                    m_tiles = []
                    for J in range(n_j):
                        mt = mt_pool.tile([128, 512], BF16, name="mt", tag="mt")
                        nc.sync.dma_start(
                            out=mt[:, :CH],
                            in_=mt_d[J * 128 : (J + 1) * 128, cs],
                        )
                        m_tiles.append(mt)
                for pr in range(4):
                    av_t = [
                        ps_av.tile([65, 512], F32, name="av", tag="av")
                        for _ in range(2)
                    ]
                    s_tiles = {}

                    def emit_S(J, pr=pr, c=c, s_tiles=s_tiles):
                        r = J - c * JT_PER_CH
                        off = max(0, r) * 128 if mask_mode == "causal" else 0
                        w = CH - off
                        spair = ps_big.tile([128, 1024], F32, name="spair", tag="big")
                        for hh in range(2):
                            hs = slice(hh * 64, (hh + 1) * 64)
                            dst = (
                                spair[:, off:CH]
                                if hh == 0
                                else spair[:, 512 : 512 + w]
                            )
                            nc.tensor.matmul(
                                dst,
                                kT_sb[hs, pr, J * 128 : (J + 1) * 128],
                                qT_sb[hs, pr, c * CH + off : (c + 1) * CH],
                                start=True,
                                stop=True,
                            )
                        s_tiles[J] = (spair, off)

                    if SKEW:
                        emit_S(0)
                    for J in range(n_j):
                        if SKEW:
                            if J + 1 < n_j:
                                emit_S(J + 1)
                        else:
                            emit_S(J)
                        spair, off = s_tiles.pop(J)
                        w = CH - off
                        r = J - c * JT_PER_CH
                        b_sl = [slice(off, CH), slice(512, 512 + w)]
                        e_pair = est_pool.tile([128, 1024], BF16, name="e_t", tag="e")
                        nc.scalar.activation(
                            e_pair[:, off : 512 + w],
                            spair[:, off : 512 + w],
                            Exp,
                            scale=0.125,
                        )
                        if mask_mode == "causal" and 0 <= r < JT_PER_CH:
                            for hh in range(2):
                                d0 = b_sl[hh].start
                                nc.vector.tensor_mul(
                                    e_pair[:, d0 : d0 + 128],
                                    e_pair[:, d0 : d0 + 128],
                                    tri_sb,
                                )
                        elif mask_mode == "general":
                            for hh in range(2):
                                nc.vector.tensor_mul(
                                    e_pair[:, b_sl[hh]],
                                    e_pair[:, b_sl[hh]],
                                    m_tiles[J][:, :CH],
                                )
                        for hh in range(2):
                            h = 2 * pr + hh
                            nc.tensor.matmul(
                                av_t[hh][:, off:CH],
                                v_sb[:, J, h * 65 : (h + 1) * 65],
                                e_pair[:, b_sl[hh]],
                                start=(J == 0),
                                stop=(J == n_j - 1),
                            )
                    for hh in range(2):
                        hs = slice(hh * 64, (hh + 1) * 64)
                        av = av_t[hh]
                        l_sb = small.tile([1, 512], F32, name="l_sb", tag="lsb")
                        nc.vector.tensor_copy(l_sb[:, :CH], av[64:65, :CH])
                        zraw = small.tile([64, 512], BF16, name="zraw", tag="zraw")
                        nc.vector.tensor_copy(zraw[:, :CH], av[0:64, :CH])
                        av = None
                        linv = small.tile([1, 512], F32, name="linv", tag="linv")
                        if RECIP_MODE == "approx":
                            nc.vector.reciprocal_approx_fast(
                                linv[:, :CH], l_sb[:, :CH]
                            )
                        elif RECIP_MODE == "lnexp":
                            lt = small.tile([1, 512], F32, name="lt", tag="lt")
                            nc.scalar.activation(
                                lt[:, :CH],
                                l_sb[:, :CH],
                                mybir.ActivationFunctionType.Ln,
                                scale=1.0,
                            )
                            nc.scalar.activation(
                                linv[:, :CH],
                                lt[:, :CH],
                                Exp,
                                scale=-1.0,
                            )
                        else:
                            nc.vector.reciprocal(linv[:, :CH], l_sb[:, :CH])
                        lbc = small.tile([64, 512], F32, name="lbc", tag="lbc")
                        nc.gpsimd.partition_broadcast(
                            lbc[:, :CH], linv[:, :CH], channels=64
                        )
                        nc.vector.tensor_mul(
                            concat_sb[hs, pr, cs], zraw[:, :CH], lbc[:, :CH]
                        )
                    if c > 0:
                        emit_outproj_tile((c - 1) * JT_PER_CH + pr)
                for it in range((NCH - 1) * JT_PER_CH, NCH * JT_PER_CH):
                    if c == NCH - 1:
                        emit_outproj_tile(it)
            if mask_mode == "general":
                _mt_cm.__exit__(None, None, None)
    nc.finalize()
    return nc


# ---------------------------------------------------------------------------
# Optional NTFF profiling (test.py sets TRACE=True). Registers the missing
# antenv.axon_hooks module so run_bass_kernel_spmd's trace path works.
TRACE = False
LAST_EXEC_TIME_NS = None
LAST_RESULTS = None


def _ensure_ntff_hook():
    import sys as _sys
    import types as _types

    if "antenv.axon_hooks" in _sys.modules:
        return
    mod = _types.ModuleType("antenv.axon_hooks")
    state = {"hook": None}
    mod.set_axon_ntff_profile_hook = lambda h: state.__setitem__("hook", h)
    mod.get_axon_ntff_profile_hook = lambda: state["hook"]
    _sys.modules["antenv.axon_hooks"] = mod
    import antenv

    antenv.axon_hooks = mod
    try:
        from trn_agent_boot.trn_boot import _ntff_profile_via_ctypes

        hook = _ntff_profile_via_ctypes("/opt/axon/libaxon_pjrt.so")
        if hook is not None:
            mod.set_axon_ntff_profile_hook(hook)
    except Exception:
        pass


_PROGRAM_CACHE = {}


def _get_program(T, mask_mode, has_bias):
    key = (T, mask_mode, has_bias)
    if key not in _PROGRAM_CACHE:
        if mask_mode == "causal":
            _PROGRAM_CACHE[key] = build_core_program(T, has_bias)
        else:
            _PROGRAM_CACHE[key] = build_core_program_general(T, mask_mode, has_bias)
    return _PROGRAM_CACHE[key]


def _mask_mode_of(mask):
    m = np.asarray(mask)
    if m.all():
        return "full"
    T = m.shape[0]
    tril = np.tril(np.ones((T, T), dtype=bool))
    if np.array_equal(m.astype(bool), tril):
        return "causal"
    return "general"


def kernel(x, mask, Wq, bq, Wk, bk, Wv, bv, Wo, bo):
    x = np.asarray(x)
    B, T, D_ = x.shape
    H = Wq.shape[0]
    assert D_ == D and H == 16
    mask_mode = _mask_mode_of(mask)
    has_bias = bool(
        np.any(np.asarray(bq)) or np.any(np.asarray(bk)) or np.any(np.asarray(bv))
    )
    nc = _get_program(T, mask_mode, has_bias)

    tri = np.triu(np.ones((128, 128), dtype=np.float32)).astype(nbf16)
    idn = np.eye(64, dtype=np.float32).astype(nbf16)
    if mask_mode == "general":
        maskT = np.ascontiguousarray(np.asarray(mask).T.astype(np.float32)).astype(
            nbf16
        )

    in_maps = []
    for core in range(8):
        b, g = core // 2, core % 2
        hsl = slice(g * HL, (g + 1) * HL)
        # (h, d, e) -> (d, h*e)
        wq = np.ascontiguousarray(
            np.transpose(np.asarray(Wq)[hsl], (1, 0, 2)).reshape(D, 512)
        ).astype(nbf16)
        wk = np.ascontiguousarray(
            np.transpose(np.asarray(Wk)[hsl], (1, 0, 2)).reshape(D, 512)
        ).astype(nbf16)
        wv = np.ascontiguousarray(
            np.transpose(np.asarray(Wv)[hsl], (1, 0, 2)).reshape(D, 512)
        ).astype(nbf16)
        wo = np.ascontiguousarray(np.asarray(Wo)[:, g * 512 : (g + 1) * 512].T).astype(
            nbf16
        )
        im = {
            "xT": np.ascontiguousarray(x[b].T).astype(nbf16),
            "wq": wq,
            "wk": wk,
            "wv": wv,
            "wo": wo,
            "tri": tri,
        }
        if mask_mode != "causal":
            im["idn"] = idn
        if mask_mode == "general":
            im["maskT"] = maskT
        if has_bias:
            im["wqb"] = np.asarray(bq)[hsl].reshape(1, 512).astype(nbf16)
            im["wkb"] = np.asarray(bk)[hsl].reshape(1, 512).astype(nbf16)
            im["wvb"] = np.asarray(bv)[hsl].reshape(1, 512).astype(nbf16)
        in_maps.append(im)

    global LAST_EXEC_TIME_NS, LAST_RESULTS
    if TRACE:
        _ensure_ntff_hook()
    res = run_bass_kernel_spmd(nc, in_maps, core_ids=list(range(8)), trace=TRACE)
    LAST_RESULTS = res
    if TRACE:
        LAST_EXEC_TIME_NS = res.exec_time_ns
    out = np.empty((B, T, D), dtype=np.float32)
    bo_f = np.asarray(bo, dtype=np.float32)
    for b in range(B):
        out[b] = (
            np.asarray(res.results[2 * b]["y"], dtype=np.float32)
            + np.asarray(res.results[2 * b + 1]["y"], dtype=np.float32)
            + bo_f
        )
    return out
